# revision 17
# baseline (speedup 1.0000x reference)
"""DCRNN seq2seq (encoder/decoder DCGRU, K=3 Chebyshev diffusion) on 8 NeuronCores.

Sharding: data-parallel over batch (8 batch elements per core); weights and the
200x200 support replicated; no collectives.

v3 — wavefront encoder + cached diffusions + algebraic decoder feedback:
  - Per-layer diffusion cache XH[l]: each h_l(t) is transposed and diffused
    exactly once; gates of (t,l+1) and (t+1,l) both read the cache (the
    baseline diffused each h twice).
  - Gate matmuls contract 6 K=64 terms (ready/old-state terms first so the PE
    can run while same-step dependencies resolve); ONE fused sigmoid per
    n-chunk computes r and u together into a [128,...] RU tile.
  - Decoder feedback folded algebraically: S_k(proj(h3)) = (S_k h3) @ (Wp Wg_k)
    + s_k (x) (pb Wg_k).  The cached XH[3] (with a constant s12 row 64) feeds
    the layer-0 x-terms directly; the projection itself is pure output work,
    off the critical path.  Decoder t=0 uses an unfolded bias column (the
    baseline's pb-fold was stale at t=0).
  - Encoder cells issued by wavefront diagonal (t+l) in phase waves
    (gates -> rh/diffuse -> cand -> tail/cache-diffuse) so up to 4 independent
    cells keep the tensor engine continuously busy (HAM stays un-throttled).
  - Candidate chunk-pairs col-tiled into ONE psum bank (tile_position
    (0,0)/(0,64)); term-major matmul order reuses LDWEIGHTS across n-chunks.
  - GRU tail b-half + rh-mul b-half on the (otherwise idle) GpSimd engine;
    psum evacuations round-robin Scalar/Vector.

All matmul operands bf16 (fp32 psum accumulate).
"""

import numpy as np
import ml_dtypes

import concourse.bass as bass
import concourse.tile as tile
from concourse import bacc, mybir
from concourse.bass_utils import run_bass_kernel_spmd

BF = ml_dtypes.bfloat16
F32 = np.float32

N = 200
U = 64
L = 4
T = 12
B = 64
NCORES = 8
BL = B // NCORES
M0, M1 = 128, 72
NB = 128  # n width of the 'b' half-tile (xbar transpose needs 128-col tiles)
NCH = [(0, 64), (64, 64), (128, 64), (192, 8)]
WAVE = 4  # max cells in flight per wavefront diagonal

dt = mybir.dt
AF = mybir.ActivationFunctionType

_CACHE = {}


def _build(enc_T=T, dec_T=T, wavefront=True):
    nc = bacc.Bacc()

    d = {}

    def din(name, shape, dtype=dt.bfloat16):
        d[name] = nc.dram_tensor(name, shape, dtype, kind='ExternalInput')

    din('SS0', [M0, 400])
    din('SS1', [M1, 400])
    din('Wp', [U + 1, 200])
    din('s12', [1, BL, 400])
    for p in ('e', 'd'):
        if p == 'e':
            din(p + 'g0x', [200, 3, 128])
            din(p + 'c0x', [200, 3, 64])
        din(p + 'g0h', [64, 3, 128])
        din(p + 'c0h', [64, 3, 64])
        din(p + 'gk0lo', [64, 3, 128])
        din(p + 'gk0hi', [64, 3, 128])
        din(p + 'gk12lo2', [128, 3, 128])
        din(p + 'gk12hi2', [128, 3, 128])
        din(p + 'gk12hi3', [64, 2, 128])
        din(p + 'g0h12', [128, 128])
        din(p + 'c0h12', [128, 64])
        din(p + 'cLk0x', [64, 3, 64])
        din(p + 'cLh', [64, 3, 64])
        din(p + 'cLx12', [128, 3, 64])
        din(p + 'cLrh12', [128, 3, 64])
        din(p + 'bg', [128, 5], dt.float32)
        din(p + 'bc', [128, 5], dt.float32)
    din('dWfg', [64, 128])
    din('dWfc', [64, 64])
    din('dWg12', [65, 2, 128])
    din('dWc12', [65, 2, 64])
    din('xTe', [enc_T, 2, M0, BL, 200])
    din('xfme', [enc_T, 2, M0, BL, 200])
    d['onm'] = nc.dram_tensor('onm', [max(dec_T, 1), 200, BL, 200], dt.float16,
                              kind='ExternalOutput')

    with tile.TileContext(nc) as tc:
        with (
            tc.tile_pool(name='const', bufs=1) as cp,
            tc.tile_pool(name='state', bufs=1) as sp,
            tc.tile_pool(name='work', bufs=2) as wp,
            tc.tile_pool(name='xin', bufs=2) as xp,
            tc.tile_pool(name='dps', bufs=2, space='PSUM') as dps,
            tc.tile_pool(name='gps', bufs=4, space='PSUM') as gps,
            tc.tile_pool(name='cps', bufs=2, space='PSUM') as cps,
        ):
            # ---- load constants / weights ----
            CT = {}
            for name, t_ in d.items():
                if name in ('onm', 'xTe', 'xfme'):
                    continue
                shape = list(t_.shape)
                if shape[0] == 200:  # split node-feature-major weights
                    CT[name + '@a'] = cp.tile([M0] + shape[1:], t_.dtype, name='t' + name + 'a')
                    CT[name + '@b'] = cp.tile([M1] + shape[1:], t_.dtype, name='t' + name + 'b')
                    nc.sync.dma_start(out=CT[name + '@a'], in_=t_[0:M0])
                    nc.sync.dma_start(out=CT[name + '@b'], in_=t_[M0:200])
                else:
                    CT[name] = cp.tile(shape, t_.dtype, name='t' + name)
                    nc.sync.dma_start(out=CT[name], in_=t_[:])
            SS = [CT['SS0'], CT['SS1']]
            Wp = CT['Wp']

            # ---- state (single-buffered; issue order + WAR deps serialize) --
            HA, HB, XH = [], [], []
            for l in range(L):
                r = 65 if l == 3 else 64
                HA.append(sp.tile([r, BL, 128], dt.bfloat16, name=f'HA{l}'))
                HB.append(sp.tile([r, BL, NB], dt.bfloat16, name=f'HB{l}'))
                if l == 3:
                    XH.append(sp.tile([65, BL, 400], dt.bfloat16, name=f'XH{l}'))
                    nc.vector.memset(XH[l][0:64], 0.0)
                else:
                    # k-stacked: rows 0:64 = S1*h, 64:128 = S2*h; cols = n
                    XH.append(sp.tile([128, BL, 200], dt.bfloat16, name=f'XH{l}'))
                    nc.vector.memset(XH[l][:], 0.0)
                nc.vector.memset(HA[l][:], 0.0)
                nc.vector.memset(HB[l][:], 0.0)
                if l == 3:
                    nc.vector.memset(HA[l][64:65], 1.0)
                    nc.vector.memset(HB[l][64:65], 1.0)
                    # s12 row for the decoder rank-1 bias fold
                    nc.sync.dma_start(out=XH[l][64:65], in_=d['s12'][:])

            evac_ctr = [0]

            def evac(dst, src):
                # round-robin psum evacuation across Scalar/Vector (3:2 vector)
                i = evac_ctr[0] % 5
                evac_ctr[0] += 1
                if i in (0, 2):
                    nc.scalar.copy(dst, src)
                else:
                    nc.vector.tensor_copy(dst, src)

            memset_ctr = {}

            def fresh_zero_cols(tag, bufs, sub):
                """memset pad columns only for the first `bufs` uses of a tag."""
                n = memset_ctr.get(tag, 0)
                if n < bufs:
                    nc.vector.memset(sub, 0.0)
                    memset_ctr[tag] = n + 1

            def diffuse_pair(lhs0, lhs1):
                ps = dps.tile([M0, 400], dt.float32, name='dp', tag='dps')
                nc.tensor.matmul(ps[:], lhs0, SS[0][:], start=True, stop=False)
                nc.tensor.matmul(ps[:], lhs1, SS[1][:], start=False, stop=True)
                return ps

            def diffuse_stack(HT0, HT1, b0):
                # psum [(k1 64f | k2 64f), 2b, 200n] via col-tiled pairs
                ps = dps.tile([M0, 2, 200], dt.float32, name='dp', tag='dps')
                for bb in range(2):
                    la = HT0[:, b0 + bb, :]
                    lb = HT1[0:M1, b0 + bb, :]
                    nc.tensor.matmul(ps[0:64, bb, :], la, SS[0][:, 0:200],
                                     start=True, stop=False, tile_position=(0, 0))
                    nc.tensor.matmul(ps[0:64, bb, :], lb, SS[1][:, 0:200],
                                     start=False, stop=True, tile_position=(0, 0))
                    nc.tensor.matmul(ps[64:128, bb, :], la, SS[0][:, 200:400],
                                     start=True, stop=False, tile_position=(0, 64))
                    nc.tensor.matmul(ps[64:128, bb, :], lb, SS[1][:, 200:400],
                                     start=False, stop=True, tile_position=(0, 64))
                return ps

            def k12s(Xt, ci):
                n0, nw = NCH[ci]
                return Xt[0:128, :, n0:n0 + nw]

            def fm(A, Bt, ci, rows=64):
                n0, nw = NCH[ci]
                if ci < 2:
                    return A[0:rows, :, n0:n0 + nw]
                return Bt[0:rows, :, n0 - 128:n0 - 128 + nw]

            def xs(Xt, ci, rows):
                # slice of a full-n (200-col) tile
                n0, nw = NCH[ci]
                return Xt[0:rows, :, n0:n0 + nw]

            def k12(Xt, k, ci, rows=64):
                n0, nw = NCH[ci]
                c0 = 200 * (k - 1) + n0
                return Xt[0:rows, :, c0:c0 + nw]

            CTX = {}

            # ---------------- phase 1: gates ----------------
            def p1(p, l, t):
                RFMa = wp.tile([64, BL, 128], dt.bfloat16, name='RFMa', tag='RFMa', bufs=WAVE)
                RFMb = wp.tile([64, BL, 80], dt.bfloat16, name='RFMb', tag='RFMb', bufs=WAVE)
                UFMa = wp.tile([64, BL, 128], dt.bfloat16, name='UFMa', tag='UFMa', bufs=WAVE)
                UFMb = wp.tile([64, BL, 80], dt.bfloat16, name='UFMb', tag='UFMb', bufs=WAVE)
                bias_col = l
                if l == 0:
                    g0h = CT[p + 'g0h']
                    terms = [
                        (g0h[:, 0, :], lambda ci: fm(HA[0], HB[0], ci)),
                        (CT[p + 'g0h12'][:], lambda ci: k12s(XH[0], ci)),
                    ]
                    if p == 'e':
                        xfm0, xfm1, Xga, Xgb = CTX['x', t]
                        g0xa, g0xb = CT['eg0x@a'], CT['eg0x@b']
                        terms += [
                            (g0xa[:, 0, :], lambda ci: xs(xfm0, ci, M0)),
                            (g0xb[0:M1, 0, :], lambda ci: xs(xfm1, ci, M1)),
                            (g0xa[:, 1, :], lambda ci: k12(Xga, 1, ci, M0)),
                            (g0xb[0:M1, 1, :], lambda ci: k12(Xgb, 1, ci, M1)),
                            (g0xa[:, 2, :], lambda ci: k12(Xga, 2, ci, M0)),
                            (g0xb[0:M1, 2, :], lambda ci: k12(Xgb, 2, ci, M1)),
                        ]
                    elif t > 0:
                        terms += [
                            (CT['dWfg'][:], lambda ci: fm(HA[3], HB[3], ci)),
                            (CT['dWg12'][:, 0, :], lambda ci: k12(XH[3], 1, ci, 65)),
                            (CT['dWg12'][:, 1, :], lambda ci: k12(XH[3], 2, ci, 65)),
                        ]
                    else:
                        bias_col = 4  # unfolded bias: x == 0 at decoder t=0
                else:
                    gk0lo, gk0hi = CT[p + 'gk0lo'], CT[p + 'gk0hi']
                    terms = [(gk0hi[:, l - 1, :], lambda ci: fm(HA[l], HB[l], ci))]
                    if l == 3:
                        terms += [
                            (CT[p + 'gk12hi3'][:, 0, :], lambda ci: k12(XH[3], 1, ci)),
                            (CT[p + 'gk12hi3'][:, 1, :], lambda ci: k12(XH[3], 2, ci)),
                        ]
                    else:
                        terms += [(CT[p + 'gk12hi2'][:, l - 1, :], lambda ci: k12s(XH[l], ci))]
                    terms += [
                        (gk0lo[:, l - 1, :], lambda ci: fm(HA[l - 1], HB[l - 1], ci)),
                        (CT[p + 'gk12lo2'][:, l - 1, :], lambda ci: k12s(XH[l - 1], ci)),
                    ]
                pss = [gps.tile([M0, BL, NCH[ci][1]], dt.float32, name='gp', tag='gps')
                       for ci in range(4)]
                nterm = len(terms)
                for j, (w, rhsfn) in enumerate(terms):
                    for ci in range(4):
                        nc.tensor.matmul(pss[ci][:, :, :], w, rhsfn(ci),
                                         start=(j == 0), stop=(j == nterm - 1))
                bg = CT[p + 'bg'][:, bias_col:bias_col + 1]
                for ci, (n0, nw) in enumerate(NCH):
                    if ci < 2:
                        dr = RFMa[:, :, n0:n0 + nw]
                        du = UFMa[:, :, n0:n0 + nw]
                    else:
                        dr = RFMb[:, :, n0 - 128:n0 - 128 + nw]
                        du = UFMb[:, :, n0 - 128:n0 - 128 + nw]
                    nc.scalar.activation(dr, pss[ci][0:64, :, :], AF.Sigmoid,
                                         bias=bg[0:64], scale=1.0)
                    nc.scalar.activation(du, pss[ci][64:128, :, :], AF.Sigmoid,
                                         bias=bg[64:128], scale=1.0)
                CTX['ru', l] = (RFMa, RFMb, UFMa, UFMb)

            # ---------------- phase 2a: r*h, transpose, diffuse ----------------
            def p2a(p, l, t):
                RFMa, RFMb, UFMa, UFMb = CTX['ru', l]
                RHa = wp.tile([64, BL, 128], dt.bfloat16, name='RHa', tag='RHa', bufs=WAVE)
                RHb = wp.tile([64, BL, NB], dt.bfloat16, name='RHb', tag='RHb', bufs=WAVE)
                fresh_zero_cols('RHb', WAVE, RHb[0:64, :, 72:NB])
                nc.vector.tensor_mul(RHa[:], RFMa[:], HA[l][0:64])
                nc.gpsimd.tensor_mul(RHb[0:64, :, 0:72], RFMb[0:64, :, 0:72],
                                     HB[l][0:64, :, 0:72])
                RHT0 = wp.tile([M0, BL, 64], dt.bfloat16, name='RHT0', tag='RHT0')
                RHT1 = wp.tile([NB, BL, 64], dt.bfloat16, name='RHT1', tag='RHT1')
                nc.sync.dma_start_transpose(RHT0[:], RHa[:])
                nc.sync.dma_start_transpose(RHT1[:], RHb[:])
                Xrh = wp.tile([128, BL, 200], dt.bfloat16, name='Xrh', tag='Xrh', bufs=WAVE)
                for b in range(0, BL, 2):
                    ps = diffuse_stack(RHT0, RHT1, b)
                    evac(Xrh[:, b:b + 2, :], ps[:, :, :])
                CTX['rh', l] = (RHa, RHb, Xrh)

            # ---------------- phase 2b: candidate ----------------
            def p2b(p, l, t):
                RHa, RHb, Xrh = CTX['rh', l]
                bias_col = l
                if l == 0:
                    c0h = CT[p + 'c0h']
                    terms = [
                        (c0h[:, 0, :], lambda ci: fm(RHa, RHb, ci)),
                        (CT[p + 'c0h12'][:], lambda ci: k12s(Xrh, ci)),
                    ]
                    if p == 'e':
                        xfm0, xfm1, Xga, Xgb = CTX['x', t]
                        c0xa, c0xb = CT['ec0x@a'], CT['ec0x@b']
                        terms += [
                            (c0xa[:, 0, :], lambda ci: xs(xfm0, ci, M0)),
                            (c0xb[0:M1, 0, :], lambda ci: xs(xfm1, ci, M1)),
                            (c0xa[:, 1, :], lambda ci: k12(Xga, 1, ci, M0)),
                            (c0xb[0:M1, 1, :], lambda ci: k12(Xgb, 1, ci, M1)),
                            (c0xa[:, 2, :], lambda ci: k12(Xga, 2, ci, M0)),
                            (c0xb[0:M1, 2, :], lambda ci: k12(Xgb, 2, ci, M1)),
                        ]
                    elif t > 0:
                        terms += [
                            (CT['dWfc'][:], lambda ci: fm(HA[3], HB[3], ci)),
                            (CT['dWc12'][:, 0, :], lambda ci: k12(XH[3], 1, ci, 65)),
                            (CT['dWc12'][:, 1, :], lambda ci: k12(XH[3], 2, ci, 65)),
                        ]
                    else:
                        bias_col = 4
                else:
                    cLk0x, cLh = CT[p + 'cLk0x'], CT[p + 'cLh']
                    terms = [
                        (cLk0x[:, l - 1, :], lambda ci: fm(HA[l - 1], HB[l - 1], ci)),
                        (CT[p + 'cLx12'][:, l - 1, :], lambda ci: k12s(XH[l - 1], ci)),
                        (cLh[:, l - 1, :], lambda ci: fm(RHa, RHb, ci)),
                        (CT[p + 'cLrh12'][:, l - 1, :], lambda ci: k12s(Xrh, ci)),
                    ]
                CFMa = wp.tile([64, BL, 128], dt.bfloat16, name='CFMa', tag='CFMa', bufs=WAVE)
                CFMb = wp.tile([64, BL, 80], dt.bfloat16, name='CFMb', tag='CFMb', bufs=WAVE)
                bc = CT[p + 'bc']
                nterm = len(terms)
                pss = [cps.tile([M0, BL, 64], dt.float32, name='cp', tag='cps')
                       for _ in range(2)]
                for j, (w, rhsfn) in enumerate(terms):
                    for pi, (cx, cy) in enumerate(((0, 1), (2, 3))):
                        nwx, nwy = NCH[cx][1], NCH[cy][1]
                        nc.tensor.matmul(pss[pi][0:64, :, 0:nwx], w, rhsfn(cx),
                                         start=(j == 0), stop=(j == nterm - 1),
                                         tile_position=(0, 0))
                        nc.tensor.matmul(pss[pi][64:128, :, 0:nwy], w, rhsfn(cy),
                                         start=(j == 0), stop=(j == nterm - 1),
                                         tile_position=(0, 64))
                for pi, (cx, cy) in enumerate(((0, 1), (2, 3))):
                    for half, ci in ((0, cx), (1, cy)):
                        n0, nw = NCH[ci]
                        if ci < 2:
                            dst = CFMa[0:64, :, n0:n0 + nw]
                        else:
                            dst = CFMb[0:64, :, n0 - 128:n0 - 128 + nw]
                        nc.scalar.activation(
                            dst, pss[pi][64 * half:64 * half + 64, :, 0:nw],
                            AF.Tanh, bias=bc[64 * half:64 * half + 64,
                                             bias_col:bias_col + 1], scale=1.0)
                CTX['cfm', l] = (CFMa, CFMb)

            # ---------------- phase 3: GRU tail + h transpose + cache diffuse --
            def p3(p, l, t):
                RFMa, RFMb, UFMa, UFMb = CTX.pop(('ru', l))
                CFMa, CFMb = CTX.pop(('cfm', l))
                CTX.pop(('rh', l))
                TMPa = wp.tile([64, BL, 128], dt.bfloat16, name='TMPa', tag='TMPa')
                TMPb = wp.tile([64, BL, 72], dt.bfloat16, name='TMPb', tag='TMPb', bufs=1)
                ha = HA[l][0:64]
                nc.vector.tensor_sub(TMPa[:], ha, CFMa[:])
                nc.vector.tensor_mul(TMPa[:], UFMa[:], TMPa[:])
                nc.vector.tensor_add(ha, CFMa[:], TMPa[:])
                hb = HB[l][0:64, :, 0:72]
                cb = CFMb[0:64, :, 0:72]
                nc.gpsimd.tensor_sub(TMPb[:], hb, cb)
                nc.gpsimd.tensor_mul(TMPb[:], UFMb[0:64, :, 0:72], TMPb[:])
                nc.gpsimd.tensor_add(hb, cb, TMPb[:])
                HLT0 = wp.tile([M0, BL, 64], dt.bfloat16, name='HLT0', tag='HLT0')
                HLT1 = wp.tile([NB, BL, 64], dt.bfloat16, name='HLT1', tag='HLT1')
                nc.sync.dma_start_transpose(HLT0[:], HA[l][0:64])
                nc.sync.dma_start_transpose(HLT1[:], HB[l][0:64])
                if l == 3:
                    for b in range(0, BL, 2):
                        ps = diffuse_pair(HLT0[:, b:b + 2, :], HLT1[0:M1, b:b + 2, :])
                        evac(XH[l][0:64, b, :], ps[0:64, :])
                        evac(XH[l][0:64, b + 1, :], ps[64:128, :])
                else:
                    for b in range(0, BL, 2):
                        ps = diffuse_stack(HLT0, HLT1, b)
                        evac(XH[l][:, b:b + 2, :], ps[:, :, :])

            # ---------------- encoder x: DMA + diffusion ----------------
            def x_load(t):
                x0Ta = xp.tile([M0, BL, 200], dt.bfloat16, name='x0Ta', tag='x0Ta')
                x0Tb = xp.tile([M1, BL, 200], dt.bfloat16, name='x0Tb', tag='x0Tb')
                nc.sync.dma_start(out=x0Ta, in_=d['xTe'][t, 0])
                nc.sync.dma_start(out=x0Tb, in_=d['xTe'][t, 1, 0:M1])
                xfm0 = xp.tile([M0, BL, 200], dt.bfloat16, name='xfm0', tag='xfm0')
                xfm1 = xp.tile([M1, BL, 200], dt.bfloat16, name='xfm1', tag='xfm1')
                nc.sync.dma_start(out=xfm0, in_=d['xfme'][t, 0])
                nc.sync.dma_start(out=xfm1, in_=d['xfme'][t, 1, 0:M1])
                CTX['xload', t] = (x0Ta, x0Tb, xfm0, xfm1)

            def x_diff(t):
                x0Ta, x0Tb, xfm0, xfm1 = CTX.pop(('xload', t))
                Xga = wp.tile([M0, BL, 400], dt.bfloat16, name='Xga', tag='Xga', bufs=1)
                Xgb = wp.tile([M1, BL, 400], dt.bfloat16, name='Xgb', tag='Xgb', bufs=1)
                for b in range(BL):
                    ps = dps.tile([M0, 400], dt.float32, name='dp', tag='dps')
                    nc.tensor.matmul(ps[:], x0Ta[:, b, 0:128], SS[0][:], start=True, stop=False)
                    nc.tensor.matmul(ps[:], x0Tb[0:M1, b, 0:128], SS[1][:], start=False, stop=True)
                    evac(Xga[:, b, :], ps[:, :])
                for b in range(BL):
                    ps = dps.tile([M0, 400], dt.float32, name='dp', tag='dps')
                    nc.tensor.matmul(ps[0:M1, :], x0Ta[:, b, 128:200], SS[0][:], start=True, stop=False)
                    nc.tensor.matmul(ps[0:M1, :], x0Tb[0:M1, b, 128:200], SS[1][:], start=False, stop=True)
                    evac(Xgb[0:M1, b, :], ps[0:M1, :])
                CTX['x', t] = (xfm0, xfm1, Xga, Xgb)

            # ---------------- decoder projection (pure output work) -----------
            def proj(t):
                pT = [wp.tile([M0, BL, 200], dt.float16, name='pT0', tag='pT0', bufs=1),
                      wp.tile([M1, BL, 200], dt.float16, name='pT1', tag='pT1', bufs=1)]
                for mc, M in ((0, M0), (1, M1)):
                    for half in range(4):
                        pps = cps.tile([M0, 2, 200], dt.float32, name='pp', tag='cps')
                        for bb in range(2):
                            b = half * 2 + bb
                            if mc == 0:
                                lhsT = HA[3][0:65, b, 0:M0]
                            else:
                                lhsT = HB[3][0:65, b, 0:M1]
                            nc.tensor.matmul(pps[0:M, bb, :], lhsT, Wp[:],
                                             start=True, stop=True)
                        evac(pT[mc][0:M, half * 2:half * 2 + 2, :], pps[0:M, :, :])
                nc.sync.dma_start(out=d['onm'][t, 0:M0], in_=pT[0][:])
                nc.sync.dma_start(out=d['onm'][t, M0:200], in_=pT[1][0:M1])

            # =================== encoder (wavefront) ===================
            x_load(0)
            if wavefront:
                for dg in range(enc_T + L - 1):
                    cells = [(dg - l, l) for l in range(L) if 0 <= dg - l < enc_T]
                    if dg + 1 < enc_T:
                        x_load(dg + 1)
                    if dg < enc_T:
                        x_diff(dg)
                    for (t, l) in cells:
                        p1('e', l, t)
                    for (t, l) in cells:
                        p2a('e', l, t)
                    for (t, l) in cells:
                        p2b('e', l, t)
                    for (t, l) in cells:
                        p3('e', l, t)
                        if l == 0:
                            CTX.pop(('x', t))
            else:
                for t in range(enc_T):
                    if t + 1 < enc_T:
                        x_load(t + 1)
                    x_diff(t)
                    for l in range(L):
                        p1('e', l, t)
                        p2a('e', l, t)
                        p2b('e', l, t)
                        p3('e', l, t)
                    CTX.pop(('x', t))

            # =================== decoder (sequential) ===================
            for t in range(dec_T):
                if t > 0:
                    proj(t - 1)
                for l in range(L):
                    p1('d', l, t)
                    p2a('d', l, t)
                    p2b('d', l, t)
                    p3('d', l, t)
            proj(dec_T - 1)

    nc.compile()
    return nc


# --------------------------------------------------------------------------
# host-side prep
# --------------------------------------------------------------------------

def _prep_shared(inputs):
    def bfc(x):
        return np.ascontiguousarray(np.asarray(x).astype(BF))

    S = np.asarray(inputs['support'], np.float64)
    S2 = 2.0 * (S @ S) - np.eye(N)
    SS = np.concatenate([S.astype(F32), S2.astype(F32)], axis=1)
    s12 = np.concatenate([S.sum(0), S2.sum(0)]).astype(F32)  # S symmetric
    out = {
        'SS0': bfc(SS[0:M0]),
        'SS1': bfc(SS[M0:200]),
        's12': bfc(np.broadcast_to(s12[None, None, :], (1, BL, 400))),
        'Wp': bfc(np.concatenate(
            [np.asarray(inputs['proj_W'], F32),
             np.asarray(inputs['proj_b'], F32)[None, :]], axis=0)),
    }
    for p, pre in (('e', 'enc_'), ('d', 'dec_')):
        Wg0 = np.asarray(inputs[pre + 'Wg0'], F32).reshape(264, 3, 128)
        Wc0 = np.asarray(inputs[pre + 'Wc0'], F32).reshape(264, 3, 64)
        if p == 'e':
            out[p + 'g0x'] = bfc(Wg0[0:200])
            out[p + 'c0x'] = bfc(Wc0[0:200])
        out[p + 'g0h'] = bfc(Wg0[200:264])
        out[p + 'c0h'] = bfc(Wc0[200:264])
        WgL = np.asarray(inputs[pre + 'Wg'], F32).reshape(3, 128, 3, 128)
        WcL = np.asarray(inputs[pre + 'Wc'], F32).reshape(3, 128, 3, 64)
        out[p + 'gk0lo'] = bfc(WgL[:, 0:64, 0, :].transpose(1, 0, 2))
        out[p + 'gk0hi'] = bfc(WgL[:, 64:128, 0, :].transpose(1, 0, 2))
        out[p + 'gk12lo2'] = bfc(np.concatenate(
            [WgL[:, 0:64, 1, :].transpose(1, 0, 2),
             WgL[:, 0:64, 2, :].transpose(1, 0, 2)], axis=0))
        out[p + 'gk12hi2'] = bfc(np.concatenate(
            [WgL[:, 64:128, 1, :].transpose(1, 0, 2),
             WgL[:, 64:128, 2, :].transpose(1, 0, 2)], axis=0))
        out[p + 'gk12hi3'] = bfc(WgL[2, 64:128, 1:3, :])
        out[p + 'g0h12'] = bfc(np.concatenate(
            [Wg0[200:264, 1, :], Wg0[200:264, 2, :]], axis=0))
        out[p + 'c0h12'] = bfc(np.concatenate(
            [Wc0[200:264, 1, :], Wc0[200:264, 2, :]], axis=0))
        out[p + 'cLk0x'] = bfc(WcL[:, 0:64, 0, :].transpose(1, 0, 2))
        out[p + 'cLh'] = bfc(WcL[:, 64:128, 0, :].transpose(1, 0, 2))
        out[p + 'cLx12'] = bfc(np.concatenate(
            [WcL[:, 0:64, 1, :].transpose(1, 0, 2),
             WcL[:, 0:64, 2, :].transpose(1, 0, 2)], axis=0))
        out[p + 'cLrh12'] = bfc(np.concatenate(
            [WcL[:, 64:128, 1, :].transpose(1, 0, 2),
             WcL[:, 64:128, 2, :].transpose(1, 0, 2)], axis=0))
        bg = np.zeros((128, 5), F32)
        bc = np.zeros((128, 5), F32)
        bg[:, 0] = np.asarray(inputs[pre + 'bg0'], F32)
        bc[0:64, 0] = np.asarray(inputs[pre + 'bc0'], F32)
        bgl = np.asarray(inputs[pre + 'bg'], F32)
        bcl = np.asarray(inputs[pre + 'bc'], F32)
        for l in range(1, 4):
            bg[:, l] = bgl[l - 1]
            bc[0:64, l] = bcl[l - 1]
        bg[:, 4] = bg[:, 0]
        bc[0:64, 4] = bc[0:64, 0]
        if p == 'd':
            pb = np.asarray(inputs['proj_b'], np.float64)
            Wpf = np.asarray(inputs['proj_W'], np.float64)
            bg[:, 0] += (pb @ Wg0[0:200, 0, :].astype(np.float64)).astype(F32)
            bc[0:64, 0] += (pb @ Wc0[0:200, 0, :].astype(np.float64)).astype(F32)
            out['dWfg'] = bfc((Wpf @ Wg0[0:200, 0, :].astype(np.float64)).astype(F32))
            out['dWfc'] = bfc((Wpf @ Wc0[0:200, 0, :].astype(np.float64)).astype(F32))
            dWg12 = np.zeros((65, 2, 128), F32)
            dWc12 = np.zeros((65, 2, 64), F32)
            for k in (1, 2):
                dWg12[0:64, k - 1] = (Wpf @ Wg0[0:200, k, :].astype(np.float64)).astype(F32)
                dWg12[64, k - 1] = (pb @ Wg0[0:200, k, :].astype(np.float64)).astype(F32)
                dWc12[0:64, k - 1] = (Wpf @ Wc0[0:200, k, :].astype(np.float64)).astype(F32)
                dWc12[64, k - 1] = (pb @ Wc0[0:200, k, :].astype(np.float64)).astype(F32)
            out['dWg12'] = bfc(dWg12)
            out['dWc12'] = bfc(dWc12)
        bc[64:128] = bc[0:64]
        out[p + 'bg'] = np.ascontiguousarray(bg)
        out[p + 'bc'] = np.ascontiguousarray(bc)
    return out


def _prep_core_x(x_core, enc_T):
    x = np.asarray(x_core, F32).reshape(BL, -1, N, 200)[:, :enc_T]
    xb = x.astype(BF)
    xTe = np.zeros((enc_T, 2, M0, BL, 200), BF)
    xfme = np.zeros((enc_T, 2, M0, BL, 200), BF)
    xt = xb.transpose(1, 2, 0, 3)  # (T, n, b, f)
    xTe[:, 0, :, :, :] = xt[:, 0:M0]
    xTe[:, 1, 0:M1, :, :] = xt[:, M0:200]
    xf = xb.transpose(1, 3, 0, 2)  # (T, f, b, n)
    xfme[:, 0, :, :, :] = xf[:, 0:M0]
    xfme[:, 1, 0:M1, :, :] = xf[:, M0:200]
    return xTe, xfme


def get_program(enc_T=T, dec_T=T):
    key = (enc_T, dec_T)
    if key not in _CACHE:
        _CACHE[key] = _build(enc_T, dec_T)
    return _CACHE[key]


def make_in_maps(inputs, enc_T=T):
    shared = _prep_shared(inputs)
    x = np.asarray(inputs['inputs'], F32)
    in_maps = []
    for c in range(NCORES):
        xTe, xfme = _prep_core_x(x[c * BL:(c + 1) * BL], enc_T)
        m = dict(shared)
        m['xTe'] = xTe
        m['xfme'] = xfme
        in_maps.append(m)
    return in_maps


def assemble_output(results, dec_T=T):
    out = np.empty((B, dec_T, N * 200), F32)
    for c in range(NCORES):
        onm = results[c]['onm']
        out[c * BL:(c + 1) * BL] = (
            onm[:dec_T].astype(F32).transpose(2, 0, 1, 3).reshape(BL, dec_T, N * 200))
    return out


def kernel(**inputs):
    nc = get_program()
    in_maps = make_in_maps(inputs)
    res = run_bass_kernel_spmd(nc, in_maps, list(range(NCORES))).results
    return assemble_output(res)


# revision 18
# speedup vs baseline: 1.1224x; 1.1224x over previous
"""DCRNN seq2seq (encoder/decoder DCGRU, K=3 Chebyshev diffusion) on 8 NeuronCores.

Sharding: data-parallel over batch (8 batch elements per core); weights and the
200x200 support replicated; no collectives.

Per-core layout (v2 — batched transposes + (n-chunk x batch)-batched matmuls):
  - f-major state per layer is a PAIR of tiles  Ha [64u, 8b, 128n], Hb [64u, 8b, 80n]
    (n split 0:128 / 128:208, cols 200:208 zero-pad).  This makes the f-major ->
    node-major conversion TWO xbar DMA transposes per quantity:
      in [64, 1024] -> out [128, 8b, 64u]   (dst = slot of the node-major tile)
      in [64,  640] -> out [ 80, 8b, 64u]
    (out[p,e,c] = in[c, e*P+p], matching the b-major-outer source layout).
  - Diffusion (contract over nodes): per-b lhsT from the node-major slot tiles,
    rhs = [S1 | S2] -> psum [feat, 400], evacuated bf16 to per-b diffused tiles.
    S2 = 2*S@S - I precomputed host-side.
  - Gate/candidate matmuls (contract features) batch ALL 8 b per n-chunk:
    F = nw*8 <= 512 per matmul (n-chunks 64/64/64/8); rhs APs are strided
    (k b n -> k n b) views of the f-major / diffused tiles; psum [P, nw, 8b].
  - ONE sigmoid per n-chunk computes r and u together (128 partitions) into an
    n-major RU tile [128, 208, 8]; tanh likewise into CFM [64, 208, 8].
  - GRU elementwise on VectorE per state-pair tile; in-place update of Ha/Hb.
  - Decoder projection: lhsT = [h3; ones] per-b slices, rhs = [proj_W; proj_b];
    decoder layer-0 k=0 x-term algebraically fused via dWfg/dWfc = Wp @ W0x_k0.

All matmul operands bf16 (fp32 psum accumulate).
"""

import numpy as np
import ml_dtypes

import concourse.bass as bass
import concourse.tile as tile
from concourse import bacc, mybir
from concourse.bass_utils import run_bass_kernel_spmd

BF = ml_dtypes.bfloat16
F32 = np.float32

N = 200
U = 64
L = 4
T = 12
B = 64
NCORES = 8
BL = B // NCORES
M0, M1 = 128, 72
NPAD = 208
NB = 128  # n width of the 'b' half-tile (covers n 128:256; valid to 200)
NCH = [(0, 64), (64, 64), (128, 64), (192, 8)]

dt = mybir.dt
AF = mybir.ActivationFunctionType

_CACHE = {}
DBG = False


def _r3(ap):
    # [K, b, n] view -> [K, n, b] iteration order (for matmul rhs / psum order)
    return ap.rearrange("k b n -> k n b")


def _rn(ap):
    # [K, n, b] view -> [K, b, n] iteration order (for elementwise vs b-major)
    return ap.rearrange("k n b -> k b n")


def _build(enc_T=T, dec_T=T):
    nc = bacc.Bacc()

    d = {}

    def din(name, shape, dtype=dt.bfloat16):
        d[name] = nc.dram_tensor(name, shape, dtype, kind='ExternalInput')

    din('SS0', [M0, 400])
    din('SS1', [M1, 400])
    din('Wp', [U + 1, 200])
    for p in ('e', 'd'):
        din(p + 'g0x', [200, 3, 128])
        din(p + 'g0h', [64, 3, 128])
        din(p + 'c0x', [200, 3, 64])
        din(p + 'c0h', [64, 3, 64])
        din(p + 'gL', [128, 3, 3, 128])
        din(p + 'gLh', [64, 3, 128])
        din(p + 'cLk0x', [64, 3, 64])
        din(p + 'cLh', [64, 3, 64])
        din(p + 'cLx', [64, 3, 2, 64])
        din(p + 'cLrh', [64, 3, 2, 64])
        din(p + 'bg', [128, 4], dt.float32)
        din(p + 'bc', [64, 4], dt.float32)
    din('dWfg', [64, 128])
    din('dWfc', [64, 64])
    din('xTe', [enc_T, 2, M0, BL, 200])
    din('xfme', [enc_T, 2, M0, BL, 200])
    d['onm'] = nc.dram_tensor('onm', [max(dec_T, 1), 200, BL, 200], dt.float32,
                              kind='ExternalOutput')
    dbg = {}

    def dbg_out(name, shape, dtype=dt.bfloat16):
        dbg[name] = nc.dram_tensor('dbg_' + name, shape, dtype,
                                   kind='ExternalOutput')
        return dbg[name]
    if DBG:
        for nm, sh in (('Xgh', [64, BL, 400]), ('Xga', [M0, BL, 400]),
                       ('Xgb', [M1, BL, 400]), ('RFM', [64, BL, 128]),
                       ('UFM', [64, BL, 128]), ('CFM', [64, BL, 128]),
                       ('RHa', [64, BL, 128]), ('RHb', [64, BL, NB]),
                       ('Xrh', [64, BL, 400]), ('HA0', [64, BL, 128]),
                       ('HB0', [64, BL, NB]), ('RHT0', [M0, 4, BL, 64]),
                       ('RHT1', [NB, 4, BL, 64]), ('HTW0', [M0, 4, BL, 64]),
                       ('HTW1', [NB, 4, BL, 64])):
            dbg_out(nm, sh)

    with tile.TileContext(nc) as tc:
        with (
            tc.tile_pool(name='const', bufs=1) as cp,
            tc.tile_pool(name='state', bufs=1) as sp,
            tc.tile_pool(name='work3', bufs=3) as wp3,
            tc.tile_pool(name='work', bufs=3) as wp,
            tc.tile_pool(name='work2', bufs=2) as wp2,
            tc.tile_pool(name='xin', bufs=2) as xp,
            tc.tile_pool(name='dps', bufs=3, space='PSUM') as diffps,
            tc.tile_pool(name='ops', bufs=4, space='PSUM') as gps,
            tc.tile_pool(name='opsn', bufs=1, space='PSUM') as gpsn,
        ):
            # ---- load constants / weights ----
            CT = {}
            for name, t_ in d.items():
                if name in ('onm', 'xTe', 'xfme'):
                    continue
                shape = list(t_.shape)
                if shape[0] == 200:  # split node-feature-major weights
                    CT[name + '@a'] = cp.tile([M0] + shape[1:], t_.dtype, name='t' + name + 'a')
                    CT[name + '@b'] = cp.tile([M1] + shape[1:], t_.dtype, name='t' + name + 'b')
                    nc.sync.dma_start(out=CT[name + '@a'], in_=t_[0:M0])
                    nc.sync.dma_start(out=CT[name + '@b'], in_=t_[M0:200])
                else:
                    CT[name] = cp.tile(shape, t_.dtype, name='t' + name)
                    nc.sync.dma_start(out=CT[name], in_=t_[:])
            SS = [CT['SS0'], CT['SS1']]
            Wp = CT['Wp']

            # ---- state ----
            HA = [sp.tile([64, BL, 128], dt.bfloat16, name=f'HA{i}') for i in range(3)]
            HB = [sp.tile([64, BL, NB], dt.bfloat16, name=f'HB{i}') for i in range(3)]
            HA.append(sp.tile([65, BL, 128], dt.bfloat16, name='HA3'))
            HB.append(sp.tile([65, BL, NB], dt.bfloat16, name='HB3'))
            # node-major storage, slot-OUTER so every slot is a contiguous xbar
            # transpose destination and b-adjacent pairs form one P=128 lhsT:
            #   HLT: h per layer;  RHT: r*h per layer
            HLT0 = sp.tile([M0, 4, BL, 64], dt.bfloat16, name='HLT0')
            HLT1 = sp.tile([NB, 4, BL, 64], dt.bfloat16, name='HLT1')
            RHT0 = sp.tile([M0, 4, BL, 64], dt.bfloat16, name='RHT0')
            RHT1 = sp.tile([NB, 4, BL, 64], dt.bfloat16, name='RHT1')

            for t_ in HA + HB + [HLT0, HLT1, RHT0, RHT1]:
                nc.vector.memset(t_[:], 0.0)
            nc.vector.memset(HA[3][64:65], 1.0)
            nc.vector.memset(HB[3][64:65], 1.0)

            def evac(i, dst, src):
                if i % 2 == 0:
                    nc.scalar.copy(dst, src)
                else:
                    nc.vector.tensor_copy(dst, src)

            def diffuse(rows, lhs, dst_ap, i):
                """psum[0:rows, 0:400] = [lhs.T @ S1 | lhs.T @ S2], evacuated
                (bf16) to dst_ap. lhs = per-m-chunk lhsT APs."""
                ps = diffps.tile([M0, 400], dt.float32, name='dps', tag='dps')
                nc.tensor.matmul(ps[0:rows, :], lhs[0], SS[0][:], start=True, stop=False)
                nc.tensor.matmul(ps[0:rows, :], lhs[1], SS[1][:], start=False, stop=True)
                evac(i, dst_ap, ps[0:rows, :])

            def diffuse_half(slot, dst, rows, b):
                """Diffuse the b:b+2 pair of h slot `slot` into dst[rows, b(+1), :]
                (rows = the 64-row half of the diffused-gates tile this half
                feeds).  psum rows 0:64 -> b, 64:128 -> b+1."""
                ps = diffps.tile([M0, 400], dt.float32, name='dps', tag='dps')
                nc.tensor.matmul(ps[:], HLT0[:, slot, b:b + 2, :], SS[0][:], start=True, stop=False)
                nc.tensor.matmul(ps[:], HLT1[0:M1, slot, b:b + 2, :], SS[1][:], start=False, stop=True)
                r0 = rows.start
                if r0 == 0:
                    nc.vector.tensor_copy(dst[0:64, b, :], ps[0:64, :])
                    nc.scalar.copy(dst[0:64, b + 1, :], ps[64:128, :])
                else:
                    nc.scalar.copy(dst[64:128, b, :], ps[0:64, :])
                    nc.vector.tensor_copy(dst[64:128, b + 1, :], ps[64:128, :])

            def diffuse_pair(lhs, dst, b):
                """Diffuse a b-adjacent pair of 64-wide node-major quantities in
                one P=128 matmul group; psum rows 0:64 -> b, 64:128 -> b+1."""
                ps = diffps.tile([M0, 400], dt.float32, name='dps', tag='dps')
                nc.tensor.matmul(ps[:], lhs[0], SS[0][:], start=True, stop=False)
                nc.tensor.matmul(ps[:], lhs[1], SS[1][:], start=False, stop=True)
                nc.vector.tensor_copy(dst[0:64, b, :], ps[0:64, :])
                nc.scalar.copy(dst[0:64, b + 1, :], ps[64:128, :])

            def fm(pa, pb, ci, rows=64):
                """f-major state rhs for n-chunk ci: [rows, 8b, nw] (b-major)."""
                n0, nw = NCH[ci]
                if ci < 2:
                    return pa[0:rows, :, n0:n0 + nw]
                return pb[0:rows, :, n0 - 128:n0 - 128 + nw]

            def pr(pa, pb, ci):
                """(pair-tile, local n-slice) for n-chunk ci."""
                n0, nw = NCH[ci]
                if ci < 2:
                    return pa, slice(n0, n0 + nw)
                return pb, slice(n0 - 128, n0 - 128 + nw)

            def transpose_rh(l, qa, qb):
                nc.sync.dma_start_transpose(RHT0[:, l, :, :], qa)
                nc.sync.dma_start_transpose(RHT1[:, l, :, :], qb)

            def transpose_h(l, qa, qb):
                nc.sync.dma_start_transpose(HLT0[:, l, :, :], qa)
                nc.sync.dma_start_transpose(HLT1[:, l, :, :], qb)

            cellno = [0]

            def rh_and_cand_tail(p, l, RFMa, RFMb, UFMa, UFMb, cand_terms):
                """r*h -> node-major rh slot -> diffuse -> cand matmuls -> tanh
                -> GRU tail -> h transposes."""
                RHa = wp2.tile([64, BL, 128], dt.bfloat16, name='RHa', tag='RHa')
                RHb = wp2.tile([64, BL, NB], dt.bfloat16, name='RHb', tag='RHb')
                nc.vector.tensor_mul(RHa[:], RFMa[:], HA[l][0:64])
                nc.vector.tensor_mul(RHb[0:64, :, 0:72], RFMb[0:64, :, 0:72],
                                     HB[l][0:64, :, 0:72])
                nc.vector.memset(RHb[0:64, :, 72:NB], 0.0)
                first_cell = DBG and cellno[0] == 0
                if first_cell:
                    nc.sync.dma_start(out=dbg['RFM'][:], in_=RFMa[:])
                    nc.sync.dma_start(out=dbg['UFM'][:], in_=UFMa[:])
                    nc.sync.dma_start(out=dbg['RHa'][:], in_=RHa[:])
                    nc.sync.dma_start(out=dbg['RHb'][:], in_=RHb[:])
                transpose_rh(l, RHa[:], RHb[:])
                Xrh = wp2.tile([64, BL, 400], dt.bfloat16, name='Xrh', tag='Xh')
                for b in range(0, BL, 2):
                    diffuse_pair([RHT0[:, l, b:b + 2, :], RHT1[0:M1, l, b:b + 2, :]],
                                 Xrh, b)
                CFMa = wp2.tile([64, BL, 128], dt.bfloat16, name='CFMa', tag='CFMa')
                CFMb = wp2.tile([64, BL, NB], dt.bfloat16, name='CFMb', tag='CFMb')
                bc = CT[p + 'bc'][:, l:l + 1]
                # col-tiled pairs: (c0,c1) share a psum bank on column groups
                # (0,0)/(0,64); (c2,c3) likewise but c3 gets its own narrow tile.
                for pi, (cx, cy) in enumerate(((0, 1), (2, 3))):
                    tx = cand_terms(cx, Xrh, RHa, RHb)
                    ty = cand_terms(cy, Xrh, RHa, RHb)
                    nwx, nwy = NCH[cx][1], NCH[cy][1]
                    psx = gps.tile([M0, BL, nwx], dt.float32, name='ops', tag='ops')
                    if nwy == nwx:
                        psy = gps.tile([M0, BL, nwy], dt.float32, name='ops', tag='ops')
                    else:
                        psy = gpsn.tile([M0, BL, nwy], dt.float32, name='opsn', tag='opsn')
                    nterm = len(tx)
                    for j, ((wx, rx), (wy, ry)) in enumerate(zip(tx, ty)):
                        first, last = j == 0, j == nterm - 1
                        nc.tensor.matmul(psx[0:64, :, :], wx, rx,
                                         start=first, stop=last,
                                         tile_position=(0, 0))
                        nc.tensor.matmul(psy[64:128, :, :], wy, ry,
                                         start=first, stop=last,
                                         tile_position=(0, 64))
                    ctx_, slx = pr(CFMa, CFMb, cx)
                    cty_, sly = pr(CFMa, CFMb, cy)
                    nc.scalar.activation(ctx_[0:64, :, slx], psx[0:64, :, :],
                                         AF.Tanh, bias=bc, scale=1.0)
                    nc.scalar.activation(cty_[0:64, :, sly], psy[64:128, :, :],
                                         AF.Tanh, bias=bc, scale=1.0)
                # ---- GRU tail:  h = c + u*(h - c)  (in place, per pair tile) ----
                TMPa = wp2.tile([64, BL, 128], dt.bfloat16, name='TMPa', tag='TMPa')
                TMPb = wp2.tile([64, BL, NB], dt.bfloat16, name='TMPb', tag='TMPb')
                ha = HA[l][0:64]
                hb = HB[l][0:64, :, 0:72]
                ca = CFMa[:]
                cb = CFMb[0:64, :, 0:72]
                nc.vector.tensor_sub(TMPa[:], ha, ca)
                nc.vector.tensor_mul(TMPa[:], UFMa[:], TMPa[:])
                nc.vector.tensor_add(ha, ca, TMPa[:])
                tb = TMPb[0:64, :, 0:72]
                nc.vector.tensor_sub(tb, hb, cb)
                nc.vector.tensor_mul(tb, UFMb[0:64, :, 0:72], tb)
                nc.vector.tensor_add(hb, cb, tb)
                transpose_h(l, HA[l][0:64], HB[l][0:64])
                if first_cell:
                    nc.sync.dma_start(out=dbg['Xrh'][:], in_=Xrh[:])
                    nc.sync.dma_start(out=dbg['CFM'][:], in_=CFMa[:])
                    nc.sync.dma_start(out=dbg['HA0'][:], in_=HA[l][0:64])
                    nc.sync.dma_start(out=dbg['HB0'][:], in_=HB[l][0:64])
                    nc.sync.dma_start(out=dbg['RHT0'][:], in_=RHT0[:])
                    nc.sync.dma_start(out=dbg['RHT1'][:], in_=RHT1[:])
                    nc.sync.dma_start(out=dbg['HTW0'][:], in_=HLT0[:])
                    nc.sync.dma_start(out=dbg['HTW1'][:], in_=HLT1[:])
                cellno[0] += 1

            def cell_upper(p, l):
                gL, gLh = CT[p + 'gL'], CT[p + 'gLh']
                cLk0x, cLh = CT[p + 'cLk0x'], CT[p + 'cLh']
                cLx, cLrh = CT[p + 'cLx'], CT[p + 'cLrh']
                # -- gates: diffuse [h_{l-1} | h_l] (contiguous slot window) --
                Xg = wp3.tile([M0, BL, 400], dt.bfloat16, name='Xg', tag='Xg')
                # h_l(t-1) half is available since the previous step: pure
                # gap-filler work.  h_{l-1}(t) half is on the dependency chain.
                for b in range(0, BL, 2):
                    diffuse_half(l, Xg, slice(64, 128), b)
                for b in range(0, BL, 2):
                    diffuse_half(l - 1, Xg, slice(0, 64), b)
                RFMa = wp.tile([64, BL, 128], dt.bfloat16, name='RFMa', tag='RFMa')
                RFMb = wp2.tile([64, BL, NB], dt.bfloat16, name='RFMb', tag='RFMb')
                UFMa = wp.tile([64, BL, 128], dt.bfloat16, name='UFMa', tag='UFMa')
                UFMb = wp2.tile([64, BL, NB], dt.bfloat16, name='UFMb', tag='UFMb')
                bg = CT[p + 'bg'][:, l:l + 1]
                for ci, (n0, nw) in enumerate(NCH):
                    ps = gps.tile([M0, BL, nw], dt.float32, name='ops', tag='ops')
                    o = ps[:, :, :]
                    nc.tensor.matmul(o, gL[0:64, l - 1, 0, :], fm(HA[l - 1], HB[l - 1], ci), start=True, stop=False)
                    nc.tensor.matmul(o, gLh[:, l - 1, :], fm(HA[l], HB[l], ci), start=False, stop=False)
                    nc.tensor.matmul(o, gL[:, l - 1, 1, :], Xg[:, :, n0:n0 + nw], start=False, stop=False)
                    nc.tensor.matmul(o, gL[:, l - 1, 2, :], Xg[:, :, 200 + n0:200 + n0 + nw], start=False, stop=True)
                    rt_, sl = pr(RFMa, RFMb, ci)
                    ut_, _ = pr(UFMa, UFMb, ci)
                    nc.scalar.activation(rt_[:, :, sl], ps[0:64, :, :],
                                         AF.Sigmoid, bias=bg[0:64], scale=1.0)
                    nc.scalar.activation(ut_[:, :, sl], ps[64:128, :, :],
                                         AF.Sigmoid, bias=bg[64:128], scale=1.0)

                def cand_terms(ci, Xrh, RHa, RHb):
                    n0, nw = NCH[ci]
                    return [
                        (cLk0x[:, l - 1, :], fm(HA[l - 1], HB[l - 1], ci)),
                        (cLx[:, l - 1, 0, :], Xg[0:64, :, n0:n0 + nw]),
                        (cLx[:, l - 1, 1, :], Xg[0:64, :, 200 + n0:200 + n0 + nw]),
                        (cLh[:, l - 1, :], fm(RHa, RHb, ci)),
                        (cLrh[:, l - 1, 0, :], Xrh[0:64, :, n0:n0 + nw]),
                        (cLrh[:, l - 1, 1, :], Xrh[0:64, :, 200 + n0:200 + n0 + nw]),
                    ]

                rh_and_cand_tail(p, l, RFMa, RFMb, UFMa, UFMb, cand_terms)

            def cell0(p, x_terms, x0Ta, x0Tb, xfm0, xfm1):
                enc = (p == 'e')
                g0xa, g0xb, g0h = CT[p + 'g0x@a'], CT[p + 'g0x@b'], CT[p + 'g0h']
                c0xa, c0xb, c0h = CT[p + 'c0x@a'], CT[p + 'c0x@b'], CT[p + 'c0h']
                if x_terms:
                    Xga = wp3.tile([M0, BL, 400], dt.bfloat16, name='Xga', tag='Xg')
                    Xgb = wp.tile([M1, BL, 400], dt.bfloat16, name='Xgb', tag='Xgb')
                    for b in range(BL):
                        diffuse(128, [x0Ta[:, b, 0:128],
                                      x0Tb[0:M1, b, 0:128]], Xga[:, b, :], b)
                    for b in range(BL):
                        diffuse(M1, [x0Ta[:, b, 128:200],
                                     x0Tb[0:M1, b, 128:200]], Xgb[0:M1, b, :], b)
                Xgh = wp2.tile([64, BL, 400], dt.bfloat16, name='Xgh', tag='Xh')
                for b in range(0, BL, 2):
                    diffuse_pair([HLT0[:, 0, b:b + 2, :], HLT1[0:M1, 0, b:b + 2, :]],
                                 Xgh, b)
                if DBG and cellno[0] == 0:
                    nc.sync.dma_start(out=dbg['Xgh'][:], in_=Xgh[:])
                    if x_terms:
                        nc.sync.dma_start(out=dbg['Xga'][:], in_=Xga[:])
                        nc.sync.dma_start(out=dbg['Xgb'][:], in_=Xgb[0:M1])
                RFMa = wp.tile([64, BL, 128], dt.bfloat16, name='RFMa', tag='RFMa')
                RFMb = wp2.tile([64, BL, NB], dt.bfloat16, name='RFMb', tag='RFMb')
                UFMa = wp.tile([64, BL, 128], dt.bfloat16, name='UFMa', tag='UFMa')
                UFMb = wp2.tile([64, BL, NB], dt.bfloat16, name='UFMb', tag='UFMb')
                bg = CT[p + 'bg'][:, 0:1]
                for ci, (n0, nw) in enumerate(NCH):
                    ps = gps.tile([M0, BL, nw], dt.float32, name='ops', tag='ops')
                    o = ps[:, :, :]
                    first = True
                    if x_terms:
                        if enc:
                            nc.tensor.matmul(o, g0xa[:, 0, :], xfm0[:, :, n0:n0 + nw], start=True, stop=False)
                            nc.tensor.matmul(o, g0xb[0:M1, 0, :], xfm1[0:M1, :, n0:n0 + nw], start=False, stop=False)
                        else:
                            nc.tensor.matmul(o, CT['dWfg'][:], fm(HA[3], HB[3], ci), start=True, stop=False)
                        for k in (1, 2):
                            s = slice(200 * (k - 1) + n0, 200 * (k - 1) + n0 + nw)
                            nc.tensor.matmul(o, g0xa[:, k, :], Xga[:, :, s], start=False, stop=False)
                            nc.tensor.matmul(o, g0xb[0:M1, k, :], Xgb[0:M1, :, s], start=False, stop=False)
                        first = False
                    nc.tensor.matmul(o, g0h[:, 0, :], fm(HA[0], HB[0], ci), start=first, stop=False)
                    nc.tensor.matmul(o, g0h[:, 1, :], Xgh[0:64, :, n0:n0 + nw], start=False, stop=False)
                    nc.tensor.matmul(o, g0h[:, 2, :], Xgh[0:64, :, 200 + n0:200 + n0 + nw], start=False, stop=True)
                    rt_, sl = pr(RFMa, RFMb, ci)
                    ut_, _ = pr(UFMa, UFMb, ci)
                    nc.scalar.activation(rt_[:, :, sl], ps[0:64, :, :],
                                         AF.Sigmoid, bias=bg[0:64], scale=1.0)
                    nc.scalar.activation(ut_[:, :, sl], ps[64:128, :, :],
                                         AF.Sigmoid, bias=bg[64:128], scale=1.0)

                def cand_terms(ci, Xch, RHa, RHb):
                    n0, nw = NCH[ci]
                    terms = []
                    if x_terms:
                        if enc:
                            terms += [(c0xa[:, 0, :], xfm0[:, :, n0:n0 + nw]),
                                      (c0xb[0:M1, 0, :], xfm1[0:M1, :, n0:n0 + nw])]
                        else:
                            terms += [(CT['dWfc'][:], fm(HA[3], HB[3], ci))]
                        for k in (1, 2):
                            s = slice(200 * (k - 1) + n0, 200 * (k - 1) + n0 + nw)
                            terms += [(c0xa[:, k, :], Xga[:, :, s]),
                                      (c0xb[0:M1, k, :], Xgb[0:M1, :, s])]
                    terms += [(c0h[:, 0, :], fm(RHa, RHb, ci)),
                              (c0h[:, 1, :], Xch[0:64, :, n0:n0 + nw]),
                              (c0h[:, 2, :], Xch[0:64, :, 200 + n0:200 + n0 + nw])]
                    return terms

                rh_and_cand_tail(p, 0, RFMa, RFMb, UFMa, UFMb, cand_terms)

            # ---- encoder ----
            for t in range(enc_T):
                x0Ta = xp.tile([M0, BL, 200], dt.bfloat16, name='x0Ta', tag='x0Ta')
                x0Tb = xp.tile([M1, BL, 200], dt.bfloat16, name='x0Tb', tag='x0Tb')
                nc.sync.dma_start(out=x0Ta, in_=d['xTe'][t, 0])
                nc.sync.dma_start(out=x0Tb, in_=d['xTe'][t, 1, 0:M1])
                xfm0 = xp.tile([M0, BL, 200], dt.bfloat16, name='xfm0', tag='xfm0')
                xfm1 = xp.tile([M1, BL, 200], dt.bfloat16, name='xfm1', tag='xfm1')
                nc.sync.dma_start(out=xfm0, in_=d['xfme'][t, 0])
                nc.sync.dma_start(out=xfm1, in_=d['xfme'][t, 1, 0:M1])
                cell0('e', True, x0Ta, x0Tb, xfm0, xfm1)
                for l in range(1, L):
                    cell_upper('e', l)

            # ---- decoder ----
            x0Ta = x0Tb = None
            for t in range(dec_T):
                cell0('d', t > 0, x0Ta, x0Tb, None, None)
                for l in range(1, L):
                    cell_upper('d', l)
                pT = [wp2.tile([M0, BL, 200], dt.float32, name='pT0', tag='pT0'),
                      wp2.tile([M1, BL, 200], dt.float32, name='pT1', tag='pT1')]
                for mc, M, Hx in ((0, M0, None), (1, M1, None)):
                    for half in range(4):
                        pps = gps.tile([M0, 2, 200], dt.float32, name='ops', tag='ops')
                        for bb in range(2):
                            b = half * 2 + bb
                            if mc == 0:
                                lhsT = HA[3][0:65, b, 0:M0]
                            else:
                                lhsT = HB[3][0:65, b, 0:M1]
                            nc.tensor.matmul(pps[0:M, bb, :], lhsT, Wp[:],
                                             start=True, stop=True)
                        evac(half, pT[mc][0:M, half * 2:half * 2 + 2, :],
                             pps[0:M, :, :])
                nc.sync.dma_start(out=d['onm'][t, 0:M0], in_=pT[0][:])
                nc.sync.dma_start(out=d['onm'][t, M0:200], in_=pT[1][0:M1])
                if t < dec_T - 1:
                    x0Ta = xp.tile([M0, BL, 200], dt.bfloat16, name='x0Ta', tag='x0Ta')
                    x0Tb = xp.tile([M1, BL, 200], dt.bfloat16, name='x0Tb', tag='x0Tb')
                    nc.vector.tensor_copy(x0Ta[:], pT[0][:])
                    nc.vector.tensor_copy(x0Tb[:], pT[1][0:M1])

    nc.compile()
    return nc


# --------------------------------------------------------------------------
# host-side prep
# --------------------------------------------------------------------------

def _prep_shared(inputs):
    def bfc(x):
        return np.ascontiguousarray(np.asarray(x).astype(BF))

    S = np.asarray(inputs['support'], np.float64)
    S2 = 2.0 * (S @ S) - np.eye(N)
    SS = np.concatenate([S.astype(F32), S2.astype(F32)], axis=1)
    out = {
        'SS0': bfc(SS[0:M0]),
        'SS1': bfc(SS[M0:200]),
        'Wp': bfc(np.concatenate(
            [np.asarray(inputs['proj_W'], F32),
             np.asarray(inputs['proj_b'], F32)[None, :]], axis=0)),
    }
    for p, pre in (('e', 'enc_'), ('d', 'dec_')):
        Wg0 = np.asarray(inputs[pre + 'Wg0'], F32).reshape(264, 3, 128)
        Wc0 = np.asarray(inputs[pre + 'Wc0'], F32).reshape(264, 3, 64)
        out[p + 'g0x'] = bfc(Wg0[0:200])
        out[p + 'g0h'] = bfc(Wg0[200:264])
        out[p + 'c0x'] = bfc(Wc0[0:200])
        out[p + 'c0h'] = bfc(Wc0[200:264])
        WgL = np.asarray(inputs[pre + 'Wg'], F32).reshape(3, 128, 3, 128)
        WcL = np.asarray(inputs[pre + 'Wc'], F32).reshape(3, 128, 3, 64)
        out[p + 'gL'] = bfc(WgL.transpose(1, 0, 2, 3))          # (c, l-1, k, o)
        out[p + 'gLh'] = bfc(WgL[:, 64:128, 0, :].transpose(1, 0, 2))
        out[p + 'cLk0x'] = bfc(WcL[:, 0:64, 0, :].transpose(1, 0, 2))
        out[p + 'cLh'] = bfc(WcL[:, 64:128, 0, :].transpose(1, 0, 2))
        out[p + 'cLx'] = bfc(WcL[:, 0:64, 1:3, :].transpose(1, 0, 2, 3))
        out[p + 'cLrh'] = bfc(WcL[:, 64:128, 1:3, :].transpose(1, 0, 2, 3))
        bg = np.zeros((128, 4), F32)
        bc = np.zeros((64, 4), F32)
        bg[:, 0] = np.asarray(inputs[pre + 'bg0'], F32)
        bc[:, 0] = np.asarray(inputs[pre + 'bc0'], F32)
        bgl = np.asarray(inputs[pre + 'bg'], F32)
        bcl = np.asarray(inputs[pre + 'bc'], F32)
        for l in range(1, 4):
            bg[:, l] = bgl[l - 1]
            bc[:, l] = bcl[l - 1]
        if p == 'd':
            pb = np.asarray(inputs['proj_b'], np.float64)
            bg[:, 0] += (pb @ Wg0[0:200, 0, :].astype(np.float64)).astype(F32)
            bc[:, 0] += (pb @ Wc0[0:200, 0, :].astype(np.float64)).astype(F32)
            Wpf = np.asarray(inputs['proj_W'], np.float64)
            out['dWfg'] = bfc((Wpf @ Wg0[0:200, 0, :].astype(np.float64)).astype(F32))
            out['dWfc'] = bfc((Wpf @ Wc0[0:200, 0, :].astype(np.float64)).astype(F32))
        out[p + 'bg'] = np.ascontiguousarray(bg)
        out[p + 'bc'] = np.ascontiguousarray(bc)
    return out


def _prep_core_x(x_core, enc_T):
    x = np.asarray(x_core, F32).reshape(BL, -1, N, 200)[:, :enc_T]
    xb = x.astype(BF)
    xTe = np.zeros((enc_T, 2, M0, BL, 200), BF)
    xfme = np.zeros((enc_T, 2, M0, BL, 200), BF)
    xt = xb.transpose(1, 2, 0, 3)  # (T, n, b, f)
    xTe[:, 0, :, :, :] = xt[:, 0:M0]
    xTe[:, 1, 0:M1, :, :] = xt[:, M0:200]
    xf = xb.transpose(1, 3, 0, 2)  # (T, f, b, n)
    xfme[:, 0, :, :, :] = xf[:, 0:M0]
    xfme[:, 1, 0:M1, :, :] = xf[:, M0:200]
    return xTe, xfme


def get_program(enc_T=T, dec_T=T):
    key = (enc_T, dec_T)
    if key not in _CACHE:
        _CACHE[key] = _build(enc_T, dec_T)
    return _CACHE[key]


def make_in_maps(inputs, enc_T=T):
    shared = _prep_shared(inputs)
    x = np.asarray(inputs['inputs'], F32)
    in_maps = []
    for c in range(NCORES):
        xTe, xfme = _prep_core_x(x[c * BL:(c + 1) * BL], enc_T)
        m = dict(shared)
        m['xTe'] = xTe
        m['xfme'] = xfme
        in_maps.append(m)
    return in_maps


def assemble_output(results, dec_T=T):
    out = np.empty((B, dec_T, N * 200), F32)
    for c in range(NCORES):
        onm = results[c]['onm']
        out[c * BL:(c + 1) * BL] = (
            onm[:dec_T].transpose(2, 0, 1, 3).reshape(BL, dec_T, N * 200))
    return out


def kernel(**inputs):
    nc = get_program()
    in_maps = make_in_maps(inputs)
    res = run_bass_kernel_spmd(nc, in_maps, list(range(NCORES))).results
    return assemble_output(res)


# revision 20
# speedup vs baseline: 1.3778x; 1.2275x over previous
"""DCRNN seq2seq (encoder/decoder DCGRU, K=3 Chebyshev diffusion) on 8 NeuronCores.

Sharding: data-parallel over batch (8 batch elements per core); weights and the
200x200 support replicated; no collectives.

v3 — wavefront encoder + cached diffusions + algebraic decoder feedback:
  - Per-layer diffusion cache XH[l]: each h_l(t) is transposed and diffused
    exactly once; gates of (t,l+1) and (t+1,l) both read the cache (the
    baseline diffused each h twice).
  - Gate matmuls contract 6 K=64 terms (ready/old-state terms first so the PE
    can run while same-step dependencies resolve); ONE fused sigmoid per
    n-chunk computes r and u together into a [128,...] RU tile.
  - Decoder feedback folded algebraically: S_k(proj(h3)) = (S_k h3) @ (Wp Wg_k)
    + s_k (x) (pb Wg_k).  The cached XH[3] (with a constant s12 row 64) feeds
    the layer-0 x-terms directly; the projection itself is pure output work,
    off the critical path.  Decoder t=0 uses an unfolded bias column (the
    baseline's pb-fold was stale at t=0).
  - Encoder cells issued by wavefront diagonal (t+l) in phase waves
    (gates -> rh/diffuse -> cand -> tail/cache-diffuse) so up to 4 independent
    cells keep the tensor engine continuously busy (HAM stays un-throttled).
  - Candidate chunk-pairs col-tiled into ONE psum bank (tile_position
    (0,0)/(0,64)); term-major matmul order reuses LDWEIGHTS across n-chunks.
  - GRU tail b-half + rh-mul b-half on the (otherwise idle) GpSimd engine;
    psum evacuations round-robin Scalar/Vector.

All matmul operands bf16 (fp32 psum accumulate).
"""

import numpy as np
import ml_dtypes

import concourse.bass as bass
import concourse.tile as tile
from concourse import bacc, mybir
from concourse.bass_utils import run_bass_kernel_spmd

BF = ml_dtypes.bfloat16
F32 = np.float32

N = 200
U = 64
L = 4
T = 12
B = 64
NCORES = 8
BL = B // NCORES
M0, M1 = 128, 72
NB = 128  # n width of the 'b' half-tile (xbar transpose needs 128-col tiles)
NCH = [(0, 64), (64, 64), (128, 64), (192, 8)]
WAVE = 4  # max cells in flight per wavefront diagonal

dt = mybir.dt
AF = mybir.ActivationFunctionType

_CACHE = {}


def _build(enc_T=T, dec_T=T, wavefront=True):
    nc = bacc.Bacc()

    d = {}

    def din(name, shape, dtype=dt.bfloat16):
        d[name] = nc.dram_tensor(name, shape, dtype, kind='ExternalInput')

    din('SS0', [M0, 400])
    din('SS1', [M1, 400])
    din('Wp', [U + 1, 200])
    din('s12', [1, BL, 400])
    for p in ('e', 'd'):
        if p == 'e':
            din(p + 'g0x', [200, 3, 128])
            din(p + 'c0x', [200, 3, 64])
        din(p + 'g0h', [64, 3, 128])
        din(p + 'c0h', [64, 3, 64])
        din(p + 'gk0lo', [64, 3, 128])
        din(p + 'gk0hi', [64, 3, 128])
        din(p + 'gk12lo2', [128, 3, 128])
        din(p + 'gk12hi2', [128, 3, 128])
        din(p + 'gk12hi3', [64, 2, 128])
        din(p + 'g0h12', [128, 128])
        din(p + 'c0h12', [128, 64])
        din(p + 'cLk0x', [64, 3, 64])
        din(p + 'cLh', [64, 3, 64])
        din(p + 'cLx12', [128, 3, 64])
        din(p + 'cLrh12', [128, 3, 64])
        din(p + 'bg', [128, 5], dt.float32)
        din(p + 'bc', [128, 5], dt.float32)
    din('dWfg', [64, 128])
    din('dWfc', [64, 64])
    din('dWg12', [65, 2, 128])
    din('dWc12', [65, 2, 64])
    din('xTe', [enc_T, 2, M0, BL, 200])
    din('xfme', [enc_T, 2, M0, BL, 200])
    d['onm'] = nc.dram_tensor('onm', [max(dec_T, 1), 200, BL, 200], dt.float16,
                              kind='ExternalOutput')

    with tile.TileContext(nc) as tc:
        with (
            tc.tile_pool(name='const', bufs=1) as cp,
            tc.tile_pool(name='state', bufs=1) as sp,
            tc.tile_pool(name='work', bufs=2) as wp,
            tc.tile_pool(name='xin', bufs=2) as xp,
            tc.tile_pool(name='dps', bufs=2, space='PSUM') as dps,
            tc.tile_pool(name='gps', bufs=4, space='PSUM') as gps,
            tc.tile_pool(name='cps', bufs=2, space='PSUM') as cps,
        ):
            # ---- load constants / weights ----
            CT = {}
            for name, t_ in d.items():
                if name in ('onm', 'xTe', 'xfme'):
                    continue
                shape = list(t_.shape)
                if shape[0] == 200:  # split node-feature-major weights
                    CT[name + '@a'] = cp.tile([M0] + shape[1:], t_.dtype, name='t' + name + 'a')
                    CT[name + '@b'] = cp.tile([M1] + shape[1:], t_.dtype, name='t' + name + 'b')
                    nc.sync.dma_start(out=CT[name + '@a'], in_=t_[0:M0])
                    nc.sync.dma_start(out=CT[name + '@b'], in_=t_[M0:200])
                else:
                    CT[name] = cp.tile(shape, t_.dtype, name='t' + name)
                    nc.sync.dma_start(out=CT[name], in_=t_[:])
            SS = [CT['SS0'], CT['SS1']]
            Wp = CT['Wp']

            # ---- state (single-buffered; issue order + WAR deps serialize) --
            HA, HB, XH = [], [], []
            for l in range(L):
                r = 65 if l == 3 else 64
                HA.append(sp.tile([r, BL, 128], dt.bfloat16, name=f'HA{l}'))
                HB.append(sp.tile([r, BL, NB], dt.bfloat16, name=f'HB{l}'))
                if l == 3:
                    XH.append(sp.tile([65, BL, 400], dt.bfloat16, name=f'XH{l}'))
                    nc.vector.memset(XH[l][0:64], 0.0)
                else:
                    # k-stacked: rows 0:64 = S1*h, 64:128 = S2*h; cols = n
                    XH.append(sp.tile([128, BL, 200], dt.bfloat16, name=f'XH{l}'))
                    nc.vector.memset(XH[l][:], 0.0)
                nc.vector.memset(HA[l][:], 0.0)
                nc.vector.memset(HB[l][:], 0.0)
                if l == 3:
                    nc.vector.memset(HA[l][64:65], 1.0)
                    nc.vector.memset(HB[l][64:65], 1.0)
                    # s12 row for the decoder rank-1 bias fold
                    nc.sync.dma_start(out=XH[l][64:65], in_=d['s12'][:])

            evac_ctr = [0]

            def evac(dst, src):
                # round-robin psum evacuation across Scalar/Vector (3:2 vector)
                i = evac_ctr[0] % 5
                evac_ctr[0] += 1
                if i in (0, 2):
                    nc.scalar.copy(dst, src)
                else:
                    nc.vector.tensor_copy(dst, src)

            memset_ctr = {}

            def fresh_zero_cols(tag, bufs, sub):
                """memset pad columns only for the first `bufs` uses of a tag."""
                n = memset_ctr.get(tag, 0)
                if n < bufs:
                    nc.vector.memset(sub, 0.0)
                    memset_ctr[tag] = n + 1

            def diffuse_pair(lhs0, lhs1):
                ps = dps.tile([M0, 400], dt.float32, name='dp', tag='dps')
                nc.tensor.matmul(ps[:], lhs0, SS[0][:], start=True, stop=False)
                nc.tensor.matmul(ps[:], lhs1, SS[1][:], start=False, stop=True)
                return ps

            def diffuse_stack(HT0, HT1, b0):
                # psum [(k1 64f | k2 64f), 2b, 200n] via col-tiled pairs
                ps = dps.tile([M0, 2, 200], dt.float32, name='dp', tag='dps')
                for bb in range(2):
                    la = HT0[:, b0 + bb, :]
                    lb = HT1[0:M1, b0 + bb, :]
                    nc.tensor.matmul(ps[0:64, bb, :], la, SS[0][:, 0:200],
                                     start=True, stop=False, tile_position=(0, 0))
                    nc.tensor.matmul(ps[0:64, bb, :], lb, SS[1][:, 0:200],
                                     start=False, stop=True, tile_position=(0, 0))
                    nc.tensor.matmul(ps[64:128, bb, :], la, SS[0][:, 200:400],
                                     start=True, stop=False, tile_position=(0, 64))
                    nc.tensor.matmul(ps[64:128, bb, :], lb, SS[1][:, 200:400],
                                     start=False, stop=True, tile_position=(0, 64))
                return ps

            def k12s(Xt, ci):
                n0, nw = NCH[ci]
                return Xt[0:128, :, n0:n0 + nw]

            def fm(A, Bt, ci, rows=64):
                n0, nw = NCH[ci]
                if ci < 2:
                    return A[0:rows, :, n0:n0 + nw]
                return Bt[0:rows, :, n0 - 128:n0 - 128 + nw]

            def xs(Xt, ci, rows):
                # slice of a full-n (200-col) tile
                n0, nw = NCH[ci]
                return Xt[0:rows, :, n0:n0 + nw]

            def k12(Xt, k, ci, rows=64):
                n0, nw = NCH[ci]
                c0 = 200 * (k - 1) + n0
                return Xt[0:rows, :, c0:c0 + nw]

            CTX = {}
            FULL = slice(0, BL)

            # ---------------- phase 1: gates ----------------
            def p1(p, l, t, bs=FULL):
                nb = bs.stop - bs.start
                RFMa = wp.tile([64, BL, 128], dt.bfloat16, name='RFMa', tag='RFMa', bufs=WAVE)
                RFMb = wp.tile([64, BL, 80], dt.bfloat16, name='RFMb', tag='RFMb', bufs=WAVE)
                UFMa = wp.tile([64, BL, 128], dt.bfloat16, name='UFMa', tag='UFMa', bufs=WAVE)
                UFMb = wp.tile([64, BL, 80], dt.bfloat16, name='UFMb', tag='UFMb', bufs=WAVE)
                bias_col = l

                def fmb(A, Bt, ci, rows=64):
                    n0, nw = NCH[ci]
                    if ci < 2:
                        return A[0:rows, bs, n0:n0 + nw]
                    return Bt[0:rows, bs, n0 - 128:n0 - 128 + nw]

                def xsb(Xt, ci, rows):
                    n0, nw = NCH[ci]
                    return Xt[0:rows, bs, n0:n0 + nw]

                def k12b(Xt, k, ci, rows=64):
                    n0, nw = NCH[ci]
                    c0 = 200 * (k - 1) + n0
                    return Xt[0:rows, bs, c0:c0 + nw]

                def k12sb(Xt, ci):
                    n0, nw = NCH[ci]
                    return Xt[0:128, bs, n0:n0 + nw]

                if l == 0:
                    g0h = CT[p + 'g0h']
                    terms = [
                        (g0h[:, 0, :], lambda ci: fmb(HA[0], HB[0], ci)),
                        (CT[p + 'g0h12'][:], lambda ci: k12sb(XH[0], ci)),
                    ]
                    if p == 'e':
                        xfm0, xfm1, Xga, Xgb = CTX['x', t]
                        g0xa, g0xb = CT['eg0x@a'], CT['eg0x@b']
                        terms += [
                            (g0xa[:, 0, :], lambda ci: xsb(xfm0, ci, M0)),
                            (g0xb[0:M1, 0, :], lambda ci: xsb(xfm1, ci, M1)),
                            (g0xa[:, 1, :], lambda ci: k12b(Xga, 1, ci, M0)),
                            (g0xb[0:M1, 1, :], lambda ci: k12b(Xgb, 1, ci, M1)),
                            (g0xa[:, 2, :], lambda ci: k12b(Xga, 2, ci, M0)),
                            (g0xb[0:M1, 2, :], lambda ci: k12b(Xgb, 2, ci, M1)),
                        ]
                    elif t > 0:
                        terms += [
                            (CT['dWfg'][:], lambda ci: fmb(HA[3], HB[3], ci)),
                            (CT['dWg12'][:, 0, :], lambda ci: k12b(XH[3], 1, ci, 65)),
                            (CT['dWg12'][:, 1, :], lambda ci: k12b(XH[3], 2, ci, 65)),
                        ]
                    else:
                        bias_col = 4  # unfolded bias: x == 0 at decoder t=0
                else:
                    gk0lo, gk0hi = CT[p + 'gk0lo'], CT[p + 'gk0hi']
                    terms = [(gk0hi[:, l - 1, :], lambda ci: fmb(HA[l], HB[l], ci))]
                    if l == 3:
                        terms += [
                            (CT[p + 'gk12hi3'][:, 0, :], lambda ci: k12b(XH[3], 1, ci)),
                            (CT[p + 'gk12hi3'][:, 1, :], lambda ci: k12b(XH[3], 2, ci)),
                        ]
                    else:
                        terms += [(CT[p + 'gk12hi2'][:, l - 1, :], lambda ci: k12sb(XH[l], ci))]
                    terms += [
                        (gk0lo[:, l - 1, :], lambda ci: fmb(HA[l - 1], HB[l - 1], ci)),
                        (CT[p + 'gk12lo2'][:, l - 1, :], lambda ci: k12sb(XH[l - 1], ci)),
                    ]
                pss = [gps.tile([M0, BL, NCH[ci][1]], dt.float32, name='gp', tag='gps')
                       for ci in range(4)]
                nterm = len(terms)
                for j, (w, rhsfn) in enumerate(terms):
                    for ci in range(4):
                        nc.tensor.matmul(pss[ci][:, 0:nb, :], w, rhsfn(ci),
                                         start=(j == 0), stop=(j == nterm - 1))
                bg = CT[p + 'bg'][:, bias_col:bias_col + 1]
                for ci, (n0, nw) in enumerate(NCH):
                    if ci < 2:
                        dr = RFMa[:, 0:nb, n0:n0 + nw]
                        du = UFMa[:, 0:nb, n0:n0 + nw]
                    else:
                        dr = RFMb[:, 0:nb, n0 - 128:n0 - 128 + nw]
                        du = UFMb[:, 0:nb, n0 - 128:n0 - 128 + nw]
                    nc.scalar.activation(dr, pss[ci][0:64, 0:nb, :], AF.Sigmoid,
                                         bias=bg[0:64], scale=1.0)
                    nc.scalar.activation(du, pss[ci][64:128, 0:nb, :], AF.Sigmoid,
                                         bias=bg[64:128], scale=1.0)
                CTX['ru', l, bs.start] = (RFMa, RFMb, UFMa, UFMb)

            # ---------------- phase 2a: r*h, transpose, diffuse ----------------
            def p2a(p, l, t, bs=FULL):
                nb = bs.stop - bs.start
                RFMa, RFMb, UFMa, UFMb = CTX['ru', l, bs.start]
                RHa = wp.tile([64, BL, 128], dt.bfloat16, name='RHa', tag='RHa', bufs=WAVE)
                RHb = wp.tile([64, BL, NB], dt.bfloat16, name='RHb', tag='RHb', bufs=WAVE)
                fresh_zero_cols('RHb', WAVE, RHb[0:64, :, 72:NB])
                nc.vector.tensor_mul(RHa[0:64, 0:nb, :], RFMa[0:64, 0:nb, :],
                                     HA[l][0:64, bs, :])
                nc.vector.tensor_mul(RHb[0:64, 0:nb, 0:72], RFMb[0:64, 0:nb, 0:72],
                                     HB[l][0:64, bs, 0:72])
                RHT0 = wp.tile([M0, BL, 64], dt.bfloat16, name='RHT0', tag='RHT0')
                RHT1 = wp.tile([NB, BL, 64], dt.bfloat16, name='RHT1', tag='RHT1')
                nc.sync.dma_start_transpose(RHT0[:, 0:nb, :], RHa[0:64, 0:nb, :])
                nc.sync.dma_start_transpose(RHT1[:, 0:nb, :], RHb[0:64, 0:nb, :])
                Xrh = wp.tile([128, BL, 200], dt.bfloat16, name='Xrh', tag='Xrh', bufs=WAVE)
                for b in range(0, nb, 2):
                    ps = diffuse_stack(RHT0, RHT1, b)
                    evac(Xrh[:, b:b + 2, :], ps[:, :, :])
                CTX['rh', l, bs.start] = (RHa, RHb, Xrh)

            # ---------------- phase 2b: candidate ----------------
            def p2b(p, l, t, bs=FULL):
                nb = bs.stop - bs.start
                RHa, RHb, Xrh = CTX['rh', l, bs.start]
                bias_col = l

                def fmb(A, Bt, ci, rows=64):
                    n0, nw = NCH[ci]
                    if ci < 2:
                        return A[0:rows, bs, n0:n0 + nw]
                    return Bt[0:rows, bs, n0 - 128:n0 - 128 + nw]

                def fml(A, Bt, ci, rows=64):
                    # local-b work tiles (RH / Xrh): rows 0:nb
                    n0, nw = NCH[ci]
                    if ci < 2:
                        return A[0:rows, 0:nb, n0:n0 + nw]
                    return Bt[0:rows, 0:nb, n0 - 128:n0 - 128 + nw]

                def xsb(Xt, ci, rows):
                    n0, nw = NCH[ci]
                    return Xt[0:rows, bs, n0:n0 + nw]

                def k12b(Xt, k, ci, rows=64):
                    n0, nw = NCH[ci]
                    c0 = 200 * (k - 1) + n0
                    return Xt[0:rows, bs, c0:c0 + nw]

                def k12sb(Xt, ci):
                    n0, nw = NCH[ci]
                    return Xt[0:128, bs, n0:n0 + nw]

                def k12sl(Xt, ci):
                    n0, nw = NCH[ci]
                    return Xt[0:128, 0:nb, n0:n0 + nw]

                if l == 0:
                    c0h = CT[p + 'c0h']
                    terms = [
                        (c0h[:, 0, :], lambda ci: fml(RHa, RHb, ci)),
                        (CT[p + 'c0h12'][:], lambda ci: k12sl(Xrh, ci)),
                    ]
                    if p == 'e':
                        xfm0, xfm1, Xga, Xgb = CTX['x', t]
                        c0xa, c0xb = CT['ec0x@a'], CT['ec0x@b']
                        terms += [
                            (c0xa[:, 0, :], lambda ci: xsb(xfm0, ci, M0)),
                            (c0xb[0:M1, 0, :], lambda ci: xsb(xfm1, ci, M1)),
                            (c0xa[:, 1, :], lambda ci: k12b(Xga, 1, ci, M0)),
                            (c0xb[0:M1, 1, :], lambda ci: k12b(Xgb, 1, ci, M1)),
                            (c0xa[:, 2, :], lambda ci: k12b(Xga, 2, ci, M0)),
                            (c0xb[0:M1, 2, :], lambda ci: k12b(Xgb, 2, ci, M1)),
                        ]
                    elif t > 0:
                        terms += [
                            (CT['dWfc'][:], lambda ci: fmb(HA[3], HB[3], ci)),
                            (CT['dWc12'][:, 0, :], lambda ci: k12b(XH[3], 1, ci, 65)),
                            (CT['dWc12'][:, 1, :], lambda ci: k12b(XH[3], 2, ci, 65)),
                        ]
                    else:
                        bias_col = 4
                else:
                    cLk0x, cLh = CT[p + 'cLk0x'], CT[p + 'cLh']
                    terms = [
                        (cLk0x[:, l - 1, :], lambda ci: fmb(HA[l - 1], HB[l - 1], ci)),
                        (CT[p + 'cLx12'][:, l - 1, :], lambda ci: k12sb(XH[l - 1], ci)),
                        (cLh[:, l - 1, :], lambda ci: fml(RHa, RHb, ci)),
                        (CT[p + 'cLrh12'][:, l - 1, :], lambda ci: k12sl(Xrh, ci)),
                    ]
                CFMa = wp.tile([64, BL, 128], dt.bfloat16, name='CFMa', tag='CFMa', bufs=WAVE)
                CFMb = wp.tile([64, BL, 80], dt.bfloat16, name='CFMb', tag='CFMb', bufs=WAVE)
                bc = CT[p + 'bc']
                nterm = len(terms)
                pss = [cps.tile([M0, BL, 64], dt.float32, name='cp', tag='cps')
                       for _ in range(2)]
                for j, (w, rhsfn) in enumerate(terms):
                    for pi, (cx, cy) in enumerate(((0, 1), (2, 3))):
                        nwx, nwy = NCH[cx][1], NCH[cy][1]
                        nc.tensor.matmul(pss[pi][0:64, 0:nb, 0:nwx], w, rhsfn(cx),
                                         start=(j == 0), stop=(j == nterm - 1),
                                         tile_position=(0, 0))
                        nc.tensor.matmul(pss[pi][64:128, 0:nb, 0:nwy], w, rhsfn(cy),
                                         start=(j == 0), stop=(j == nterm - 1),
                                         tile_position=(0, 64))
                for pi, (cx, cy) in enumerate(((0, 1), (2, 3))):
                    for half, ci in ((0, cx), (1, cy)):
                        n0, nw = NCH[ci]
                        if ci < 2:
                            dst = CFMa[0:64, 0:nb, n0:n0 + nw]
                        else:
                            dst = CFMb[0:64, 0:nb, n0 - 128:n0 - 128 + nw]
                        nc.scalar.activation(
                            dst, pss[pi][64 * half:64 * half + 64, 0:nb, 0:nw],
                            AF.Tanh, bias=bc[64 * half:64 * half + 64,
                                             bias_col:bias_col + 1], scale=1.0)
                CTX['cfm', l, bs.start] = (CFMa, CFMb)

            # ---------------- phase 3: GRU tail + h transpose + cache diffuse --
            def p3(p, l, t, bs=FULL):
                nb = bs.stop - bs.start
                RFMa, RFMb, UFMa, UFMb = CTX.pop(('ru', l, bs.start))
                CFMa, CFMb = CTX.pop(('cfm', l, bs.start))
                CTX.pop(('rh', l, bs.start))
                TMPa = wp.tile([64, BL, 128], dt.bfloat16, name='TMPa', tag='TMPa')
                TMPb = wp.tile([64, BL, 72], dt.bfloat16, name='TMPb', tag='TMPb', bufs=1)
                ha = HA[l][0:64, bs, :]
                ca = CFMa[0:64, 0:nb, :]
                ta = TMPa[0:64, 0:nb, :]
                nc.vector.tensor_sub(ta, ha, ca)
                nc.vector.tensor_mul(ta, UFMa[0:64, 0:nb, :], ta)
                nc.vector.tensor_add(ha, ca, ta)
                hb = HB[l][0:64, bs, 0:72]
                cb = CFMb[0:64, 0:nb, 0:72]
                tb = TMPb[0:64, 0:nb, :]
                nc.vector.tensor_sub(tb, hb, cb)
                nc.vector.tensor_mul(tb, UFMb[0:64, 0:nb, 0:72], tb)
                nc.vector.tensor_add(hb, cb, tb)
                HLT0 = wp.tile([M0, BL, 64], dt.bfloat16, name='HLT0', tag='HLT0')
                HLT1 = wp.tile([NB, BL, 64], dt.bfloat16, name='HLT1', tag='HLT1')
                nc.sync.dma_start_transpose(HLT0[:, 0:nb, :], HA[l][0:64, bs, :])
                nc.sync.dma_start_transpose(HLT1[:, 0:nb, :], HB[l][0:64, bs, :])
                if l == 3:
                    for b in range(0, nb, 2):
                        ps = diffuse_pair(HLT0[:, b:b + 2, :], HLT1[0:M1, b:b + 2, :])
                        evac(XH[l][0:64, bs.start + b, :], ps[0:64, :])
                        evac(XH[l][0:64, bs.start + b + 1, :], ps[64:128, :])
                else:
                    for b in range(0, nb, 2):
                        ps = diffuse_stack(HLT0, HLT1, b)
                        evac(XH[l][:, bs.start + b:bs.start + b + 2, :], ps[:, :, :])

            # ---------------- encoder x: DMA + diffusion ----------------
            def x_load(t):
                x0Ta = xp.tile([M0, BL, 200], dt.bfloat16, name='x0Ta', tag='x0Ta')
                x0Tb = xp.tile([M1, BL, 200], dt.bfloat16, name='x0Tb', tag='x0Tb')
                nc.sync.dma_start(out=x0Ta, in_=d['xTe'][t, 0])
                nc.sync.dma_start(out=x0Tb, in_=d['xTe'][t, 1, 0:M1])
                xfm0 = xp.tile([M0, BL, 200], dt.bfloat16, name='xfm0', tag='xfm0')
                xfm1 = xp.tile([M1, BL, 200], dt.bfloat16, name='xfm1', tag='xfm1')
                nc.sync.dma_start(out=xfm0, in_=d['xfme'][t, 0])
                nc.sync.dma_start(out=xfm1, in_=d['xfme'][t, 1, 0:M1])
                CTX['xload', t] = (x0Ta, x0Tb, xfm0, xfm1)

            def x_diff(t):
                x0Ta, x0Tb, xfm0, xfm1 = CTX.pop(('xload', t))
                Xga = wp.tile([M0, BL, 400], dt.bfloat16, name='Xga', tag='Xga', bufs=1)
                Xgb = wp.tile([M1, BL, 400], dt.bfloat16, name='Xgb', tag='Xgb', bufs=1)
                for b in range(BL):
                    ps = dps.tile([M0, 400], dt.float32, name='dp', tag='dps')
                    nc.tensor.matmul(ps[:], x0Ta[:, b, 0:128], SS[0][:], start=True, stop=False)
                    nc.tensor.matmul(ps[:], x0Tb[0:M1, b, 0:128], SS[1][:], start=False, stop=True)
                    evac(Xga[:, b, :], ps[:, :])
                for b in range(BL):
                    ps = dps.tile([M0, 400], dt.float32, name='dp', tag='dps')
                    nc.tensor.matmul(ps[0:M1, :], x0Ta[:, b, 128:200], SS[0][:], start=True, stop=False)
                    nc.tensor.matmul(ps[0:M1, :], x0Tb[0:M1, b, 128:200], SS[1][:], start=False, stop=True)
                    evac(Xgb[0:M1, b, :], ps[0:M1, :])
                CTX['x', t] = (xfm0, xfm1, Xga, Xgb)

            # ---------------- decoder projection (pure output work) -----------
            def proj(t):
                pT = [wp.tile([M0, BL, 200], dt.float16, name='pT0', tag='pT0', bufs=1),
                      wp.tile([M1, BL, 200], dt.float16, name='pT1', tag='pT1', bufs=1)]
                for mc, M in ((0, M0), (1, M1)):
                    for half in range(4):
                        pps = cps.tile([M0, 2, 200], dt.float32, name='pp', tag='cps')
                        for bb in range(2):
                            b = half * 2 + bb
                            if mc == 0:
                                lhsT = HA[3][0:65, b, 0:M0]
                            else:
                                lhsT = HB[3][0:65, b, 0:M1]
                            nc.tensor.matmul(pps[0:M, bb, :], lhsT, Wp[:],
                                             start=True, stop=True)
                        evac(pT[mc][0:M, half * 2:half * 2 + 2, :], pps[0:M, :, :])
                nc.sync.dma_start(out=d['onm'][t, 0:M0], in_=pT[0][:])
                nc.sync.dma_start(out=d['onm'][t, M0:200], in_=pT[1][0:M1])

            # =================== encoder (wavefront) ===================
            x_load(0)
            for dg in range(enc_T + L - 1):
                cells = [(dg - l, l) for l in range(L) if 0 <= dg - l < enc_T]
                if dg + 1 < enc_T:
                    x_load(dg + 1)
                if dg < enc_T:
                    x_diff(dg)
                for (t, l) in cells:
                    p1('e', l, t)
                for (t, l) in cells:
                    p2a('e', l, t)
                for (t, l) in cells:
                    p2b('e', l, t)
                for (t, l) in cells:
                    p3('e', l, t)
                    if l == 0:
                        CTX.pop(('x', t))

            # ======= decoder: two independent batch streams, ladder =======
            SA, SB = slice(0, 4), slice(4, 8)
            for t in range(dec_T):
                if t > 0:
                    proj(t - 1)
                for l in range(L):
                    p1('d', l, t, SA)
                    p2a('d', l, t, SA)
                    p1('d', l, t, SB)
                    p2b('d', l, t, SA)
                    p2a('d', l, t, SB)
                    p3('d', l, t, SA)
                    p2b('d', l, t, SB)
                    p3('d', l, t, SB)
            proj(dec_T - 1)

    nc.compile()
    return nc


# --------------------------------------------------------------------------
# host-side prep
# --------------------------------------------------------------------------

def _prep_shared(inputs):
    def bfc(x):
        return np.ascontiguousarray(np.asarray(x).astype(BF))

    S = np.asarray(inputs['support'], np.float64)
    S2 = 2.0 * (S @ S) - np.eye(N)
    SS = np.concatenate([S.astype(F32), S2.astype(F32)], axis=1)
    s12 = np.concatenate([S.sum(0), S2.sum(0)]).astype(F32)  # S symmetric
    out = {
        'SS0': bfc(SS[0:M0]),
        'SS1': bfc(SS[M0:200]),
        's12': bfc(np.broadcast_to(s12[None, None, :], (1, BL, 400))),
        'Wp': bfc(np.concatenate(
            [np.asarray(inputs['proj_W'], F32),
             np.asarray(inputs['proj_b'], F32)[None, :]], axis=0)),
    }
    for p, pre in (('e', 'enc_'), ('d', 'dec_')):
        Wg0 = np.asarray(inputs[pre + 'Wg0'], F32).reshape(264, 3, 128)
        Wc0 = np.asarray(inputs[pre + 'Wc0'], F32).reshape(264, 3, 64)
        if p == 'e':
            out[p + 'g0x'] = bfc(Wg0[0:200])
            out[p + 'c0x'] = bfc(Wc0[0:200])
        out[p + 'g0h'] = bfc(Wg0[200:264])
        out[p + 'c0h'] = bfc(Wc0[200:264])
        WgL = np.asarray(inputs[pre + 'Wg'], F32).reshape(3, 128, 3, 128)
        WcL = np.asarray(inputs[pre + 'Wc'], F32).reshape(3, 128, 3, 64)
        out[p + 'gk0lo'] = bfc(WgL[:, 0:64, 0, :].transpose(1, 0, 2))
        out[p + 'gk0hi'] = bfc(WgL[:, 64:128, 0, :].transpose(1, 0, 2))
        out[p + 'gk12lo2'] = bfc(np.concatenate(
            [WgL[:, 0:64, 1, :].transpose(1, 0, 2),
             WgL[:, 0:64, 2, :].transpose(1, 0, 2)], axis=0))
        out[p + 'gk12hi2'] = bfc(np.concatenate(
            [WgL[:, 64:128, 1, :].transpose(1, 0, 2),
             WgL[:, 64:128, 2, :].transpose(1, 0, 2)], axis=0))
        out[p + 'gk12hi3'] = bfc(WgL[2, 64:128, 1:3, :])
        out[p + 'g0h12'] = bfc(np.concatenate(
            [Wg0[200:264, 1, :], Wg0[200:264, 2, :]], axis=0))
        out[p + 'c0h12'] = bfc(np.concatenate(
            [Wc0[200:264, 1, :], Wc0[200:264, 2, :]], axis=0))
        out[p + 'cLk0x'] = bfc(WcL[:, 0:64, 0, :].transpose(1, 0, 2))
        out[p + 'cLh'] = bfc(WcL[:, 64:128, 0, :].transpose(1, 0, 2))
        out[p + 'cLx12'] = bfc(np.concatenate(
            [WcL[:, 0:64, 1, :].transpose(1, 0, 2),
             WcL[:, 0:64, 2, :].transpose(1, 0, 2)], axis=0))
        out[p + 'cLrh12'] = bfc(np.concatenate(
            [WcL[:, 64:128, 1, :].transpose(1, 0, 2),
             WcL[:, 64:128, 2, :].transpose(1, 0, 2)], axis=0))
        bg = np.zeros((128, 5), F32)
        bc = np.zeros((128, 5), F32)
        bg[:, 0] = np.asarray(inputs[pre + 'bg0'], F32)
        bc[0:64, 0] = np.asarray(inputs[pre + 'bc0'], F32)
        bgl = np.asarray(inputs[pre + 'bg'], F32)
        bcl = np.asarray(inputs[pre + 'bc'], F32)
        for l in range(1, 4):
            bg[:, l] = bgl[l - 1]
            bc[0:64, l] = bcl[l - 1]
        bg[:, 4] = bg[:, 0]
        bc[0:64, 4] = bc[0:64, 0]
        if p == 'd':
            pb = np.asarray(inputs['proj_b'], np.float64)
            Wpf = np.asarray(inputs['proj_W'], np.float64)
            bg[:, 0] += (pb @ Wg0[0:200, 0, :].astype(np.float64)).astype(F32)
            bc[0:64, 0] += (pb @ Wc0[0:200, 0, :].astype(np.float64)).astype(F32)
            out['dWfg'] = bfc((Wpf @ Wg0[0:200, 0, :].astype(np.float64)).astype(F32))
            out['dWfc'] = bfc((Wpf @ Wc0[0:200, 0, :].astype(np.float64)).astype(F32))
            dWg12 = np.zeros((65, 2, 128), F32)
            dWc12 = np.zeros((65, 2, 64), F32)
            for k in (1, 2):
                dWg12[0:64, k - 1] = (Wpf @ Wg0[0:200, k, :].astype(np.float64)).astype(F32)
                dWg12[64, k - 1] = (pb @ Wg0[0:200, k, :].astype(np.float64)).astype(F32)
                dWc12[0:64, k - 1] = (Wpf @ Wc0[0:200, k, :].astype(np.float64)).astype(F32)
                dWc12[64, k - 1] = (pb @ Wc0[0:200, k, :].astype(np.float64)).astype(F32)
            out['dWg12'] = bfc(dWg12)
            out['dWc12'] = bfc(dWc12)
        bc[64:128] = bc[0:64]
        out[p + 'bg'] = np.ascontiguousarray(bg)
        out[p + 'bc'] = np.ascontiguousarray(bc)
    return out


def _prep_core_x(x_core, enc_T):
    x = np.asarray(x_core, F32).reshape(BL, -1, N, 200)[:, :enc_T]
    xb = x.astype(BF)
    xTe = np.zeros((enc_T, 2, M0, BL, 200), BF)
    xfme = np.zeros((enc_T, 2, M0, BL, 200), BF)
    xt = xb.transpose(1, 2, 0, 3)  # (T, n, b, f)
    xTe[:, 0, :, :, :] = xt[:, 0:M0]
    xTe[:, 1, 0:M1, :, :] = xt[:, M0:200]
    xf = xb.transpose(1, 3, 0, 2)  # (T, f, b, n)
    xfme[:, 0, :, :, :] = xf[:, 0:M0]
    xfme[:, 1, 0:M1, :, :] = xf[:, M0:200]
    return xTe, xfme


def get_program(enc_T=T, dec_T=T):
    key = (enc_T, dec_T)
    if key not in _CACHE:
        _CACHE[key] = _build(enc_T, dec_T)
    return _CACHE[key]


def make_in_maps(inputs, enc_T=T):
    shared = _prep_shared(inputs)
    x = np.asarray(inputs['inputs'], F32)
    in_maps = []
    for c in range(NCORES):
        xTe, xfme = _prep_core_x(x[c * BL:(c + 1) * BL], enc_T)
        m = dict(shared)
        m['xTe'] = xTe
        m['xfme'] = xfme
        in_maps.append(m)
    return in_maps


def assemble_output(results, dec_T=T):
    out = np.empty((B, dec_T, N * 200), F32)
    for c in range(NCORES):
        onm = results[c]['onm']
        out[c * BL:(c + 1) * BL] = (
            onm[:dec_T].astype(F32).transpose(2, 0, 1, 3).reshape(BL, dec_T, N * 200))
    return out


def kernel(**inputs):
    nc = get_program()
    in_maps = make_in_maps(inputs)
    res = run_bass_kernel_spmd(nc, in_maps, list(range(NCORES))).results
    return assemble_output(res)


# revision 21
# speedup vs baseline: 1.4426x; 1.0470x over previous
"""DCRNN seq2seq (encoder/decoder DCGRU, K=3 Chebyshev diffusion) on 8 NeuronCores.

Sharding: data-parallel over batch (8 batch elements per core); weights and the
200x200 support replicated; no collectives.

v3 — wavefront encoder + cached diffusions + algebraic decoder feedback:
  - Per-layer diffusion cache XH[l]: each h_l(t) is transposed and diffused
    exactly once; gates of (t,l+1) and (t+1,l) both read the cache (the
    baseline diffused each h twice).
  - Gate matmuls contract 6 K=64 terms (ready/old-state terms first so the PE
    can run while same-step dependencies resolve); ONE fused sigmoid per
    n-chunk computes r and u together into a [128,...] RU tile.
  - Decoder feedback folded algebraically: S_k(proj(h3)) = (S_k h3) @ (Wp Wg_k)
    + s_k (x) (pb Wg_k).  The cached XH[3] (with a constant s12 row 64) feeds
    the layer-0 x-terms directly; the projection itself is pure output work,
    off the critical path.  Decoder t=0 uses an unfolded bias column (the
    baseline's pb-fold was stale at t=0).
  - Encoder cells issued by wavefront diagonal (t+l) in phase waves
    (gates -> rh/diffuse -> cand -> tail/cache-diffuse) so up to 4 independent
    cells keep the tensor engine continuously busy (HAM stays un-throttled).
  - Candidate chunk-pairs col-tiled into ONE psum bank (tile_position
    (0,0)/(0,64)); term-major matmul order reuses LDWEIGHTS across n-chunks.
  - GRU tail b-half + rh-mul b-half on the (otherwise idle) GpSimd engine;
    psum evacuations round-robin Scalar/Vector.

All matmul operands bf16 (fp32 psum accumulate).
"""

import numpy as np
import ml_dtypes

import concourse.bass as bass
import concourse.tile as tile
from concourse import bacc, mybir
from concourse.bass_utils import run_bass_kernel_spmd

BF = ml_dtypes.bfloat16
F32 = np.float32

N = 200
U = 64
L = 4
T = 12
B = 64
NCORES = 8
BL = B // NCORES
M0, M1 = 128, 72
NB = 128  # n width of the 'b' half-tile (xbar transpose needs 128-col tiles)
NCH = [(0, 64), (64, 64), (128, 64), (192, 8)]
WAVE = 4  # max cells in flight per wavefront diagonal

dt = mybir.dt
AF = mybir.ActivationFunctionType

_CACHE = {}


def _build(enc_T=T, dec_T=T, wavefront=True):
    nc = bacc.Bacc()

    d = {}

    def din(name, shape, dtype=dt.bfloat16):
        d[name] = nc.dram_tensor(name, shape, dtype, kind='ExternalInput')

    din('SS0', [M0, 400])
    din('SS1', [M1, 400])
    din('Wp', [U + 1, 200])
    din('s12', [1, BL, 400])
    for p in ('e', 'd'):
        if p == 'e':
            din(p + 'g0x', [200, 3, 128])
            din(p + 'c0x', [200, 3, 64])
        din(p + 'g0h', [64, 3, 128])
        din(p + 'c0h', [64, 3, 64])
        din(p + 'gk0lo', [64, 3, 128])
        din(p + 'gk0hi', [64, 3, 128])
        din(p + 'gk12lo2', [128, 3, 128])
        din(p + 'gk12hi2', [128, 3, 128])
        din(p + 'gk12hi3', [64, 2, 128])
        din(p + 'g0h12', [128, 128])
        din(p + 'c0h12', [128, 64])
        din(p + 'cLk0x', [64, 3, 64])
        din(p + 'cLh', [64, 3, 64])
        din(p + 'cLx12', [128, 3, 64])
        din(p + 'cLrh12', [128, 3, 64])
        din(p + 'bg', [128, 5], dt.float32)
        din(p + 'bc', [128, 5], dt.float32)
    din('dWfg', [64, 128])
    din('dWfc', [64, 64])
    din('dWg12', [65, 2, 128])
    din('dWc12', [65, 2, 64])
    din('xTe', [enc_T, 2, M0, BL, 200])
    din('xfme', [enc_T, 2, M0, BL, 200])
    d['onm'] = nc.dram_tensor('onm', [max(dec_T, 1), 200, BL, 200], dt.float16,
                              kind='ExternalOutput')

    with tile.TileContext(nc) as tc:
        with (
            tc.tile_pool(name='const', bufs=1) as cp,
            tc.tile_pool(name='state', bufs=1) as sp,
            tc.tile_pool(name='work', bufs=2) as wp,
            tc.tile_pool(name='xin', bufs=2) as xp,
            tc.tile_pool(name='dps', bufs=2, space='PSUM') as dps,
            tc.tile_pool(name='gps', bufs=4, space='PSUM') as gps,
            tc.tile_pool(name='cps', bufs=2, space='PSUM') as cps,
        ):
            # ---- load constants / weights ----
            CT = {}
            for name, t_ in d.items():
                if name in ('onm', 'xTe', 'xfme'):
                    continue
                shape = list(t_.shape)
                if shape[0] == 200:  # split node-feature-major weights
                    CT[name + '@a'] = cp.tile([M0] + shape[1:], t_.dtype, name='t' + name + 'a')
                    CT[name + '@b'] = cp.tile([M1] + shape[1:], t_.dtype, name='t' + name + 'b')
                    nc.sync.dma_start(out=CT[name + '@a'], in_=t_[0:M0])
                    nc.sync.dma_start(out=CT[name + '@b'], in_=t_[M0:200])
                else:
                    CT[name] = cp.tile(shape, t_.dtype, name='t' + name)
                    nc.sync.dma_start(out=CT[name], in_=t_[:])
            SS = [CT['SS0'], CT['SS1']]
            Wp = CT['Wp']

            # ---- state (single-buffered; issue order + WAR deps serialize) --
            HA, HB, XH = [], [], []
            for l in range(L):
                r = 65 if l == 3 else 64
                HA.append(sp.tile([r, BL, 128], dt.bfloat16, name=f'HA{l}'))
                HB.append(sp.tile([r, BL, NB], dt.bfloat16, name=f'HB{l}'))
                if l == 3:
                    XH.append(sp.tile([65, BL, 400], dt.bfloat16, name=f'XH{l}'))
                    nc.vector.memset(XH[l][0:64], 0.0)
                else:
                    # k-stacked: rows 0:64 = S1*h, 64:128 = S2*h; cols = n
                    XH.append(sp.tile([128, BL, 200], dt.bfloat16, name=f'XH{l}'))
                    nc.vector.memset(XH[l][:], 0.0)
                nc.vector.memset(HA[l][:], 0.0)
                nc.vector.memset(HB[l][:], 0.0)
                if l == 3:
                    nc.vector.memset(HA[l][64:65], 1.0)
                    nc.vector.memset(HB[l][64:65], 1.0)
                    # s12 row for the decoder rank-1 bias fold
                    nc.sync.dma_start(out=XH[l][64:65], in_=d['s12'][:])

            evac_ctr = [0]

            def evac(dst, src):
                # round-robin psum evacuation across Scalar/Vector (3:2 vector)
                i = evac_ctr[0] % 5
                evac_ctr[0] += 1
                if i in (0, 2):
                    nc.scalar.copy(dst, src)
                else:
                    nc.vector.tensor_copy(dst, src)

            memset_ctr = {}

            def fresh_zero_cols(tag, bufs, sub):
                """memset pad columns only for the first `bufs` uses of a tag."""
                n = memset_ctr.get(tag, 0)
                if n < bufs:
                    nc.vector.memset(sub, 0.0)
                    memset_ctr[tag] = n + 1

            def diffuse_pair(lhs0, lhs1):
                ps = dps.tile([M0, 400], dt.float32, name='dp', tag='dps')
                nc.tensor.matmul(ps[:], lhs0, SS[0][:], start=True, stop=False)
                nc.tensor.matmul(ps[:], lhs1, SS[1][:], start=False, stop=True)
                return ps

            def diffuse_stack(HT0, HT1, b0):
                # psum [(k1 64f | k2 64f), 2b, 200n] via col-tiled pairs
                ps = dps.tile([M0, 2, 200], dt.float32, name='dp', tag='dps')
                for bb in range(2):
                    la = HT0[:, b0 + bb, :]
                    lb = HT1[0:M1, b0 + bb, :]
                    nc.tensor.matmul(ps[0:64, bb, :], la, SS[0][:, 0:200],
                                     start=True, stop=False, tile_position=(0, 0))
                    nc.tensor.matmul(ps[0:64, bb, :], lb, SS[1][:, 0:200],
                                     start=False, stop=True, tile_position=(0, 0))
                    nc.tensor.matmul(ps[64:128, bb, :], la, SS[0][:, 200:400],
                                     start=True, stop=False, tile_position=(0, 64))
                    nc.tensor.matmul(ps[64:128, bb, :], lb, SS[1][:, 200:400],
                                     start=False, stop=True, tile_position=(0, 64))
                return ps

            def k12s(Xt, ci):
                n0, nw = NCH[ci]
                return Xt[0:128, :, n0:n0 + nw]

            def fm(A, Bt, ci, rows=64):
                n0, nw = NCH[ci]
                if ci < 2:
                    return A[0:rows, :, n0:n0 + nw]
                return Bt[0:rows, :, n0 - 128:n0 - 128 + nw]

            def xs(Xt, ci, rows):
                # slice of a full-n (200-col) tile
                n0, nw = NCH[ci]
                return Xt[0:rows, :, n0:n0 + nw]

            def k12(Xt, k, ci, rows=64):
                n0, nw = NCH[ci]
                c0 = 200 * (k - 1) + n0
                return Xt[0:rows, :, c0:c0 + nw]

            CTX = {}
            FULL = slice(0, BL)

            # ---------------- phase 1: gates ----------------
            def p1(p, l, t, bs=FULL):
                nb = bs.stop - bs.start
                RFMa = wp.tile([64, BL, 128], dt.bfloat16, name='RFMa', tag='RFMa', bufs=WAVE)
                RFMb = wp.tile([64, BL, 80], dt.bfloat16, name='RFMb', tag='RFMb', bufs=WAVE)
                UFMa = wp.tile([64, BL, 128], dt.bfloat16, name='UFMa', tag='UFMa', bufs=WAVE)
                UFMb = wp.tile([64, BL, 80], dt.bfloat16, name='UFMb', tag='UFMb', bufs=WAVE)
                bias_col = l

                def fmb(A, Bt, n0, nw, rows=64):
                    if n0 < 128:
                        return A[0:rows, bs, n0:n0 + nw]
                    return Bt[0:rows, bs, n0 - 128:n0 - 128 + nw]

                def xsb(Xt, n0, nw, rows):
                    return Xt[0:rows, bs, n0:n0 + nw]

                def k12b(Xt, k, n0, nw, rows=64):
                    c0 = 200 * (k - 1) + n0
                    return Xt[0:rows, bs, c0:c0 + nw]

                def k12sb(Xt, n0, nw):
                    return Xt[0:128, bs, n0:n0 + nw]

                if l == 0:
                    g0h = CT[p + 'g0h']
                    terms = [
                        (g0h[:, 0, :], lambda n0, nw: fmb(HA[0], HB[0], n0, nw)),
                        (CT[p + 'g0h12'][:], lambda n0, nw: k12sb(XH[0], n0, nw)),
                    ]
                    if p == 'e':
                        xfm0, xfm1, Xga, Xgb = CTX['x', t]
                        g0xa, g0xb = CT['eg0x@a'], CT['eg0x@b']
                        terms += [
                            (g0xa[:, 0, :], lambda n0, nw: xsb(xfm0, n0, nw, M0)),
                            (g0xb[0:M1, 0, :], lambda n0, nw: xsb(xfm1, n0, nw, M1)),
                            (g0xa[:, 1, :], lambda n0, nw: k12b(Xga, 1, n0, nw, M0)),
                            (g0xb[0:M1, 1, :], lambda n0, nw: k12b(Xgb, 1, n0, nw, M1)),
                            (g0xa[:, 2, :], lambda n0, nw: k12b(Xga, 2, n0, nw, M0)),
                            (g0xb[0:M1, 2, :], lambda n0, nw: k12b(Xgb, 2, n0, nw, M1)),
                        ]
                    elif t > 0:
                        terms += [
                            (CT['dWfg'][:], lambda n0, nw: fmb(HA[3], HB[3], n0, nw)),
                            (CT['dWg12'][:, 0, :], lambda n0, nw: k12b(XH[3], 1, n0, nw, 65)),
                            (CT['dWg12'][:, 1, :], lambda n0, nw: k12b(XH[3], 2, n0, nw, 65)),
                        ]
                    else:
                        bias_col = 4  # unfolded bias: x == 0 at decoder t=0
                else:
                    gk0lo, gk0hi = CT[p + 'gk0lo'], CT[p + 'gk0hi']
                    terms = [(gk0hi[:, l - 1, :], lambda n0, nw: fmb(HA[l], HB[l], n0, nw))]
                    if l == 3:
                        terms += [
                            (CT[p + 'gk12hi3'][:, 0, :], lambda n0, nw: k12b(XH[3], 1, n0, nw)),
                            (CT[p + 'gk12hi3'][:, 1, :], lambda n0, nw: k12b(XH[3], 2, n0, nw)),
                        ]
                    else:
                        terms += [(CT[p + 'gk12hi2'][:, l - 1, :], lambda n0, nw: k12sb(XH[l], n0, nw))]
                    terms += [
                        (gk0lo[:, l - 1, :], lambda n0, nw: fmb(HA[l - 1], HB[l - 1], n0, nw)),
                        (CT[p + 'gk12lo2'][:, l - 1, :], lambda n0, nw: k12sb(XH[l - 1], n0, nw)),
                    ]
                chks = NCH if nb == BL else [(0, 128), (128, 72)]
                pss = [gps.tile([M0, nb, nw], dt.float32, name='gp', tag='gps')
                       for (n0, nw) in chks]
                nterm = len(terms)
                for j, (w, rhsfn) in enumerate(terms):
                    for k, (n0, nw) in enumerate(chks):
                        nc.tensor.matmul(pss[k][:, :, :], w, rhsfn(n0, nw),
                                         start=(j == 0), stop=(j == nterm - 1))
                bg = CT[p + 'bg'][:, bias_col:bias_col + 1]
                for k, (n0, nw) in enumerate(chks):
                    if n0 < 128:
                        dr = RFMa[:, 0:nb, n0:n0 + nw]
                        du = UFMa[:, 0:nb, n0:n0 + nw]
                    else:
                        dr = RFMb[:, 0:nb, n0 - 128:n0 - 128 + nw]
                        du = UFMb[:, 0:nb, n0 - 128:n0 - 128 + nw]
                    nc.scalar.activation(dr, pss[k][0:64, :, :], AF.Sigmoid,
                                         bias=bg[0:64], scale=1.0)
                    nc.scalar.activation(du, pss[k][64:128, :, :], AF.Sigmoid,
                                         bias=bg[64:128], scale=1.0)
                CTX['ru', l, bs.start] = (RFMa, RFMb, UFMa, UFMb)

            # ---------------- phase 2a: r*h, transpose, diffuse ----------------
            def p2a(p, l, t, bs=FULL):
                nb = bs.stop - bs.start
                RFMa, RFMb, UFMa, UFMb = CTX['ru', l, bs.start]
                RHa = wp.tile([64, BL, 128], dt.bfloat16, name='RHa', tag='RHa', bufs=WAVE)
                RHb = wp.tile([64, BL, NB], dt.bfloat16, name='RHb', tag='RHb', bufs=WAVE)
                fresh_zero_cols('RHb', WAVE, RHb[0:64, :, 72:NB])
                nc.vector.tensor_mul(RHa[0:64, 0:nb, :], RFMa[0:64, 0:nb, :],
                                     HA[l][0:64, bs, :])
                nc.vector.tensor_mul(RHb[0:64, 0:nb, 0:72], RFMb[0:64, 0:nb, 0:72],
                                     HB[l][0:64, bs, 0:72])
                RHT0 = wp.tile([M0, BL, 64], dt.bfloat16, name='RHT0', tag='RHT0')
                RHT1 = wp.tile([NB, BL, 64], dt.bfloat16, name='RHT1', tag='RHT1')
                nc.sync.dma_start_transpose(RHT0[:, 0:nb, :], RHa[0:64, 0:nb, :])
                nc.sync.dma_start_transpose(RHT1[:, 0:nb, :], RHb[0:64, 0:nb, :])
                Xrh = wp.tile([128, BL, 200], dt.bfloat16, name='Xrh', tag='Xrh', bufs=WAVE)
                for b in range(0, nb, 2):
                    ps = diffuse_stack(RHT0, RHT1, b)
                    evac(Xrh[:, b:b + 2, :], ps[:, :, :])
                CTX['rh', l, bs.start] = (RHa, RHb, Xrh)

            # ---------------- phase 2b: candidate ----------------
            def p2b(p, l, t, bs=FULL):
                nb = bs.stop - bs.start
                RHa, RHb, Xrh = CTX['rh', l, bs.start]
                bias_col = l

                def fmb(A, Bt, n0, nw, rows=64):
                    if n0 < 128:
                        return A[0:rows, bs, n0:n0 + nw]
                    return Bt[0:rows, bs, n0 - 128:n0 - 128 + nw]

                def fml(A, Bt, n0, nw, rows=64):
                    # local-b work tiles (RH / Xrh): rows 0:nb
                    if n0 < 128:
                        return A[0:rows, 0:nb, n0:n0 + nw]
                    return Bt[0:rows, 0:nb, n0 - 128:n0 - 128 + nw]

                def xsb(Xt, n0, nw, rows):
                    return Xt[0:rows, bs, n0:n0 + nw]

                def k12b(Xt, k, n0, nw, rows=64):
                    c0 = 200 * (k - 1) + n0
                    return Xt[0:rows, bs, c0:c0 + nw]

                def k12sb(Xt, n0, nw):
                    return Xt[0:128, bs, n0:n0 + nw]

                def k12sl(Xt, n0, nw):
                    return Xt[0:128, 0:nb, n0:n0 + nw]

                if l == 0:
                    c0h = CT[p + 'c0h']
                    terms = [
                        (c0h[:, 0, :], lambda n0, nw: fml(RHa, RHb, n0, nw)),
                        (CT[p + 'c0h12'][:], lambda n0, nw: k12sl(Xrh, n0, nw)),
                    ]
                    if p == 'e':
                        xfm0, xfm1, Xga, Xgb = CTX['x', t]
                        c0xa, c0xb = CT['ec0x@a'], CT['ec0x@b']
                        terms += [
                            (c0xa[:, 0, :], lambda n0, nw: xsb(xfm0, n0, nw, M0)),
                            (c0xb[0:M1, 0, :], lambda n0, nw: xsb(xfm1, n0, nw, M1)),
                            (c0xa[:, 1, :], lambda n0, nw: k12b(Xga, 1, n0, nw, M0)),
                            (c0xb[0:M1, 1, :], lambda n0, nw: k12b(Xgb, 1, n0, nw, M1)),
                            (c0xa[:, 2, :], lambda n0, nw: k12b(Xga, 2, n0, nw, M0)),
                            (c0xb[0:M1, 2, :], lambda n0, nw: k12b(Xgb, 2, n0, nw, M1)),
                        ]
                    elif t > 0:
                        terms += [
                            (CT['dWfc'][:], lambda n0, nw: fmb(HA[3], HB[3], n0, nw)),
                            (CT['dWc12'][:, 0, :], lambda n0, nw: k12b(XH[3], 1, n0, nw, 65)),
                            (CT['dWc12'][:, 1, :], lambda n0, nw: k12b(XH[3], 2, n0, nw, 65)),
                        ]
                    else:
                        bias_col = 4
                else:
                    cLk0x, cLh = CT[p + 'cLk0x'], CT[p + 'cLh']
                    terms = [
                        (cLk0x[:, l - 1, :], lambda n0, nw: fmb(HA[l - 1], HB[l - 1], n0, nw)),
                        (CT[p + 'cLx12'][:, l - 1, :], lambda n0, nw: k12sb(XH[l - 1], n0, nw)),
                        (cLh[:, l - 1, :], lambda n0, nw: fml(RHa, RHb, n0, nw)),
                        (CT[p + 'cLrh12'][:, l - 1, :], lambda n0, nw: k12sl(Xrh, n0, nw)),
                    ]
                CFMa = wp.tile([64, BL, 128], dt.bfloat16, name='CFMa', tag='CFMa', bufs=WAVE)
                CFMb = wp.tile([64, BL, 80], dt.bfloat16, name='CFMb', tag='CFMb', bufs=WAVE)
                bc = CT[p + 'bc']
                nterm = len(terms)
                if nb == BL:
                    pairs = [(NCH[0], NCH[1]), (NCH[2], NCH[3])]
                else:
                    pairs = [((0, 128), (128, 72))]
                pss = [cps.tile([M0, nb, cx[1]], dt.float32, name='cp', tag='cps')
                       for (cx, cy) in pairs]
                for j, (w, rhsfn) in enumerate(terms):
                    for pi, (cx, cy) in enumerate(pairs):
                        nc.tensor.matmul(pss[pi][0:64, :, 0:cx[1]], w, rhsfn(*cx),
                                         start=(j == 0), stop=(j == nterm - 1),
                                         tile_position=(0, 0))
                        nc.tensor.matmul(pss[pi][64:128, :, 0:cy[1]], w, rhsfn(*cy),
                                         start=(j == 0), stop=(j == nterm - 1),
                                         tile_position=(0, 64))
                for pi, (cx, cy) in enumerate(pairs):
                    for half, (n0, nw) in ((0, cx), (1, cy)):
                        if n0 < 128:
                            dst = CFMa[0:64, 0:nb, n0:n0 + nw]
                        else:
                            dst = CFMb[0:64, 0:nb, n0 - 128:n0 - 128 + nw]
                        nc.scalar.activation(
                            dst, pss[pi][64 * half:64 * half + 64, :, 0:nw],
                            AF.Tanh, bias=bc[64 * half:64 * half + 64,
                                             bias_col:bias_col + 1], scale=1.0)
                CTX['cfm', l, bs.start] = (CFMa, CFMb)

            # ---------------- phase 3: GRU tail + h transpose + cache diffuse --
            def p3(p, l, t, bs=FULL):
                nb = bs.stop - bs.start
                RFMa, RFMb, UFMa, UFMb = CTX.pop(('ru', l, bs.start))
                CFMa, CFMb = CTX.pop(('cfm', l, bs.start))
                CTX.pop(('rh', l, bs.start))
                TMPa = wp.tile([64, BL, 128], dt.bfloat16, name='TMPa', tag='TMPa')
                TMPb = wp.tile([64, BL, 72], dt.bfloat16, name='TMPb', tag='TMPb', bufs=1)
                ha = HA[l][0:64, bs, :]
                ca = CFMa[0:64, 0:nb, :]
                ta = TMPa[0:64, 0:nb, :]
                nc.vector.tensor_sub(ta, ha, ca)
                nc.vector.tensor_mul(ta, UFMa[0:64, 0:nb, :], ta)
                nc.vector.tensor_add(ha, ca, ta)
                hb = HB[l][0:64, bs, 0:72]
                cb = CFMb[0:64, 0:nb, 0:72]
                tb = TMPb[0:64, 0:nb, :]
                nc.vector.tensor_sub(tb, hb, cb)
                nc.vector.tensor_mul(tb, UFMb[0:64, 0:nb, 0:72], tb)
                nc.vector.tensor_add(hb, cb, tb)
                HLT0 = wp.tile([M0, BL, 64], dt.bfloat16, name='HLT0', tag='HLT0')
                HLT1 = wp.tile([NB, BL, 64], dt.bfloat16, name='HLT1', tag='HLT1')
                nc.sync.dma_start_transpose(HLT0[:, 0:nb, :], HA[l][0:64, bs, :])
                nc.sync.dma_start_transpose(HLT1[:, 0:nb, :], HB[l][0:64, bs, :])
                if l == 3:
                    for b in range(0, nb, 2):
                        ps = diffuse_pair(HLT0[:, b:b + 2, :], HLT1[0:M1, b:b + 2, :])
                        evac(XH[l][0:64, bs.start + b, :], ps[0:64, :])
                        evac(XH[l][0:64, bs.start + b + 1, :], ps[64:128, :])
                else:
                    for b in range(0, nb, 2):
                        ps = diffuse_stack(HLT0, HLT1, b)
                        evac(XH[l][:, bs.start + b:bs.start + b + 2, :], ps[:, :, :])

            # ---------------- encoder x: DMA + diffusion ----------------
            def x_load(t):
                x0Ta = xp.tile([M0, BL, 200], dt.bfloat16, name='x0Ta', tag='x0Ta')
                x0Tb = xp.tile([M1, BL, 200], dt.bfloat16, name='x0Tb', tag='x0Tb')
                nc.sync.dma_start(out=x0Ta, in_=d['xTe'][t, 0])
                nc.sync.dma_start(out=x0Tb, in_=d['xTe'][t, 1, 0:M1])
                xfm0 = xp.tile([M0, BL, 200], dt.bfloat16, name='xfm0', tag='xfm0')
                xfm1 = xp.tile([M1, BL, 200], dt.bfloat16, name='xfm1', tag='xfm1')
                nc.sync.dma_start(out=xfm0, in_=d['xfme'][t, 0])
                nc.sync.dma_start(out=xfm1, in_=d['xfme'][t, 1, 0:M1])
                CTX['xload', t] = (x0Ta, x0Tb, xfm0, xfm1)

            def x_diff(t):
                x0Ta, x0Tb, xfm0, xfm1 = CTX.pop(('xload', t))
                Xga = wp.tile([M0, BL, 400], dt.bfloat16, name='Xga', tag='Xga', bufs=1)
                Xgb = wp.tile([M1, BL, 400], dt.bfloat16, name='Xgb', tag='Xgb', bufs=1)
                for b in range(BL):
                    ps = dps.tile([M0, 400], dt.float32, name='dp', tag='dps')
                    nc.tensor.matmul(ps[:], x0Ta[:, b, 0:128], SS[0][:], start=True, stop=False)
                    nc.tensor.matmul(ps[:], x0Tb[0:M1, b, 0:128], SS[1][:], start=False, stop=True)
                    evac(Xga[:, b, :], ps[:, :])
                for b in range(BL):
                    ps = dps.tile([M0, 400], dt.float32, name='dp', tag='dps')
                    nc.tensor.matmul(ps[0:M1, :], x0Ta[:, b, 128:200], SS[0][:], start=True, stop=False)
                    nc.tensor.matmul(ps[0:M1, :], x0Tb[0:M1, b, 128:200], SS[1][:], start=False, stop=True)
                    evac(Xgb[0:M1, b, :], ps[0:M1, :])
                CTX['x', t] = (xfm0, xfm1, Xga, Xgb)

            # ---------------- decoder projection (pure output work) -----------
            def proj(t):
                pT = [wp.tile([M0, BL, 200], dt.float16, name='pT0', tag='pT0', bufs=1),
                      wp.tile([M1, BL, 200], dt.float16, name='pT1', tag='pT1', bufs=1)]
                for mc, M in ((0, M0), (1, M1)):
                    for half in range(4):
                        pps = cps.tile([M0, 2, 200], dt.float32, name='pp', tag='cps')
                        for bb in range(2):
                            b = half * 2 + bb
                            if mc == 0:
                                lhsT = HA[3][0:65, b, 0:M0]
                            else:
                                lhsT = HB[3][0:65, b, 0:M1]
                            nc.tensor.matmul(pps[0:M, bb, :], lhsT, Wp[:],
                                             start=True, stop=True)
                        evac(pT[mc][0:M, half * 2:half * 2 + 2, :], pps[0:M, :, :])
                nc.sync.dma_start(out=d['onm'][t, 0:M0], in_=pT[0][:])
                nc.sync.dma_start(out=d['onm'][t, M0:200], in_=pT[1][0:M1])

            # =================== encoder (wavefront) ===================
            x_load(0)
            for dg in range(enc_T + L - 1):
                cells = [(dg - l, l) for l in range(L) if 0 <= dg - l < enc_T]
                if dg + 1 < enc_T:
                    x_load(dg + 1)
                if dg < enc_T:
                    x_diff(dg)
                for (t, l) in cells:
                    p1('e', l, t)
                for (t, l) in cells:
                    p2a('e', l, t)
                for (t, l) in cells:
                    p2b('e', l, t)
                for (t, l) in cells:
                    p3('e', l, t)
                    if l == 0:
                        CTX.pop(('x', t))

            # ======= decoder: two independent batch streams, ladder =======
            SA, SB = slice(0, 4), slice(4, 8)
            for t in range(dec_T):
                if t > 0:
                    proj(t - 1)
                for l in range(L):
                    p1('d', l, t, SA)
                    p2a('d', l, t, SA)
                    p1('d', l, t, SB)
                    p2b('d', l, t, SA)
                    p2a('d', l, t, SB)
                    p3('d', l, t, SA)
                    p2b('d', l, t, SB)
                    p3('d', l, t, SB)
            proj(dec_T - 1)

    nc.compile()
    return nc


# --------------------------------------------------------------------------
# host-side prep
# --------------------------------------------------------------------------

def _prep_shared(inputs):
    def bfc(x):
        return np.ascontiguousarray(np.asarray(x).astype(BF))

    S = np.asarray(inputs['support'], np.float64)
    S2 = 2.0 * (S @ S) - np.eye(N)
    SS = np.concatenate([S.astype(F32), S2.astype(F32)], axis=1)
    s12 = np.concatenate([S.sum(0), S2.sum(0)]).astype(F32)  # S symmetric
    out = {
        'SS0': bfc(SS[0:M0]),
        'SS1': bfc(SS[M0:200]),
        's12': bfc(np.broadcast_to(s12[None, None, :], (1, BL, 400))),
        'Wp': bfc(np.concatenate(
            [np.asarray(inputs['proj_W'], F32),
             np.asarray(inputs['proj_b'], F32)[None, :]], axis=0)),
    }
    for p, pre in (('e', 'enc_'), ('d', 'dec_')):
        Wg0 = np.asarray(inputs[pre + 'Wg0'], F32).reshape(264, 3, 128)
        Wc0 = np.asarray(inputs[pre + 'Wc0'], F32).reshape(264, 3, 64)
        if p == 'e':
            out[p + 'g0x'] = bfc(Wg0[0:200])
            out[p + 'c0x'] = bfc(Wc0[0:200])
        out[p + 'g0h'] = bfc(Wg0[200:264])
        out[p + 'c0h'] = bfc(Wc0[200:264])
        WgL = np.asarray(inputs[pre + 'Wg'], F32).reshape(3, 128, 3, 128)
        WcL = np.asarray(inputs[pre + 'Wc'], F32).reshape(3, 128, 3, 64)
        out[p + 'gk0lo'] = bfc(WgL[:, 0:64, 0, :].transpose(1, 0, 2))
        out[p + 'gk0hi'] = bfc(WgL[:, 64:128, 0, :].transpose(1, 0, 2))
        out[p + 'gk12lo2'] = bfc(np.concatenate(
            [WgL[:, 0:64, 1, :].transpose(1, 0, 2),
             WgL[:, 0:64, 2, :].transpose(1, 0, 2)], axis=0))
        out[p + 'gk12hi2'] = bfc(np.concatenate(
            [WgL[:, 64:128, 1, :].transpose(1, 0, 2),
             WgL[:, 64:128, 2, :].transpose(1, 0, 2)], axis=0))
        out[p + 'gk12hi3'] = bfc(WgL[2, 64:128, 1:3, :])
        out[p + 'g0h12'] = bfc(np.concatenate(
            [Wg0[200:264, 1, :], Wg0[200:264, 2, :]], axis=0))
        out[p + 'c0h12'] = bfc(np.concatenate(
            [Wc0[200:264, 1, :], Wc0[200:264, 2, :]], axis=0))
        out[p + 'cLk0x'] = bfc(WcL[:, 0:64, 0, :].transpose(1, 0, 2))
        out[p + 'cLh'] = bfc(WcL[:, 64:128, 0, :].transpose(1, 0, 2))
        out[p + 'cLx12'] = bfc(np.concatenate(
            [WcL[:, 0:64, 1, :].transpose(1, 0, 2),
             WcL[:, 0:64, 2, :].transpose(1, 0, 2)], axis=0))
        out[p + 'cLrh12'] = bfc(np.concatenate(
            [WcL[:, 64:128, 1, :].transpose(1, 0, 2),
             WcL[:, 64:128, 2, :].transpose(1, 0, 2)], axis=0))
        bg = np.zeros((128, 5), F32)
        bc = np.zeros((128, 5), F32)
        bg[:, 0] = np.asarray(inputs[pre + 'bg0'], F32)
        bc[0:64, 0] = np.asarray(inputs[pre + 'bc0'], F32)
        bgl = np.asarray(inputs[pre + 'bg'], F32)
        bcl = np.asarray(inputs[pre + 'bc'], F32)
        for l in range(1, 4):
            bg[:, l] = bgl[l - 1]
            bc[0:64, l] = bcl[l - 1]
        bg[:, 4] = bg[:, 0]
        bc[0:64, 4] = bc[0:64, 0]
        if p == 'd':
            pb = np.asarray(inputs['proj_b'], np.float64)
            Wpf = np.asarray(inputs['proj_W'], np.float64)
            bg[:, 0] += (pb @ Wg0[0:200, 0, :].astype(np.float64)).astype(F32)
            bc[0:64, 0] += (pb @ Wc0[0:200, 0, :].astype(np.float64)).astype(F32)
            out['dWfg'] = bfc((Wpf @ Wg0[0:200, 0, :].astype(np.float64)).astype(F32))
            out['dWfc'] = bfc((Wpf @ Wc0[0:200, 0, :].astype(np.float64)).astype(F32))
            dWg12 = np.zeros((65, 2, 128), F32)
            dWc12 = np.zeros((65, 2, 64), F32)
            for k in (1, 2):
                dWg12[0:64, k - 1] = (Wpf @ Wg0[0:200, k, :].astype(np.float64)).astype(F32)
                dWg12[64, k - 1] = (pb @ Wg0[0:200, k, :].astype(np.float64)).astype(F32)
                dWc12[0:64, k - 1] = (Wpf @ Wc0[0:200, k, :].astype(np.float64)).astype(F32)
                dWc12[64, k - 1] = (pb @ Wc0[0:200, k, :].astype(np.float64)).astype(F32)
            out['dWg12'] = bfc(dWg12)
            out['dWc12'] = bfc(dWc12)
        bc[64:128] = bc[0:64]
        out[p + 'bg'] = np.ascontiguousarray(bg)
        out[p + 'bc'] = np.ascontiguousarray(bc)
    return out


def _prep_core_x(x_core, enc_T):
    x = np.asarray(x_core, F32).reshape(BL, -1, N, 200)[:, :enc_T]
    xb = x.astype(BF)
    xTe = np.zeros((enc_T, 2, M0, BL, 200), BF)
    xfme = np.zeros((enc_T, 2, M0, BL, 200), BF)
    xt = xb.transpose(1, 2, 0, 3)  # (T, n, b, f)
    xTe[:, 0, :, :, :] = xt[:, 0:M0]
    xTe[:, 1, 0:M1, :, :] = xt[:, M0:200]
    xf = xb.transpose(1, 3, 0, 2)  # (T, f, b, n)
    xfme[:, 0, :, :, :] = xf[:, 0:M0]
    xfme[:, 1, 0:M1, :, :] = xf[:, M0:200]
    return xTe, xfme


def get_program(enc_T=T, dec_T=T):
    key = (enc_T, dec_T)
    if key not in _CACHE:
        _CACHE[key] = _build(enc_T, dec_T)
    return _CACHE[key]


def make_in_maps(inputs, enc_T=T):
    shared = _prep_shared(inputs)
    x = np.asarray(inputs['inputs'], F32)
    in_maps = []
    for c in range(NCORES):
        xTe, xfme = _prep_core_x(x[c * BL:(c + 1) * BL], enc_T)
        m = dict(shared)
        m['xTe'] = xTe
        m['xfme'] = xfme
        in_maps.append(m)
    return in_maps


def assemble_output(results, dec_T=T):
    out = np.empty((B, dec_T, N * 200), F32)
    for c in range(NCORES):
        onm = results[c]['onm']
        out[c * BL:(c + 1) * BL] = (
            onm[:dec_T].astype(F32).transpose(2, 0, 1, 3).reshape(BL, dec_T, N * 200))
    return out


def kernel(**inputs):
    nc = get_program()
    in_maps = make_in_maps(inputs)
    res = run_bass_kernel_spmd(nc, in_maps, list(range(NCORES))).results
    return assemble_output(res)
